# revision 1
# baseline (speedup 1.0000x reference)
"""Trainium2 Bass kernel for nn_DetectModel (RGAT x3 + TopK pooling + MLP head).

Structure exploited (validated vs reference):
  - x_l[v] = emb[a_v] * c_l(a_v): pooling only scales node features by
    attr-dependent tanh scores, so all per-edge quantities come from tiny
    tables indexed by s = t*10 + a_src (1140) and c2 = a_dst*114 + t (1140).
  - P_e = exp(lrelu(Tq[c2] + Tk[s])) = max(EAq[c2]*EAk[s], E2Aq[c2]*E2Ak[s])
    with pre-exponentiated tables; the segment max cancels in the softmax.
  - Node v survives pool l  <=>  rank_a(v) < k_l[a_v]  (k_l computed on
    device from tanh scores + host-known attr counts).
  - Sharding: dst-vertex sharding into 64 degree-balanced buckets
    (8 NeuronCores x 8 GPSIMD core-groups); segment reductions never cross
    cores. ELL layout (k-th in-edge per vertex, vertices sorted by degree)
    turns segment sums into identity-matmul PSUM accumulations.

Device pipeline per layer: GPSIMD ap_gather (tables, per-core-broadcast) ->
DVE elementwise (products, mask) -> PE identity-matmul segment accumulation
into PSUM (U, S) -> per-vertex tail (recip, relu+bias, alive mask, reduce).
Final: one AllGather of [128, 8] partials, replicated tiny MLP, out[1].
"""
import numpy as np

import concourse.bass as bass
import concourse.bacc as bacc
import concourse.mybir as mybir
import concourse.tile as tile
from concourse.bass_utils import run_bass_kernel_spmd
from concourse.masks import make_identity

F32 = mybir.dt.float32
I16 = mybir.dt.int16
AF = mybir.ActivationFunctionType
OP = mybir.AluOpType
AX = mybir.AxisListType

N0, N1, N2 = 50000, 40000, 32000
E = 600000
D = 16
R = 114
NA = 10
NCORES = 8
G = 8
NB = NCORES * G

NE_TAB = 1152          # table slots (>= 1141), dead index = 1140
DEAD_S = 1140
DEAD_C2 = 1140
QUOTAS = (N0, N1, N2)
TQ3 = ((0, 380), (380, 760), (760, 1140))


# ---------------------------------------------------------------- host prep

def host_prep(node_ids, edge_index, edge_type):
    a = np.asarray(node_ids).astype(np.int64)
    src = np.asarray(edge_index)[0].astype(np.int64)
    dst = np.asarray(edge_index)[1].astype(np.int64)
    t = np.asarray(edge_type).astype(np.int64)
    n0 = a.shape[0]

    cnt = np.bincount(a, minlength=NA).astype(np.int64)
    order_a = np.argsort(a, kind="stable")
    rank = np.empty(n0, np.int64)
    starts = np.zeros(NA + 1, np.int64)
    np.cumsum(cnt, out=starts[1:])
    rank[order_a] = np.arange(n0) - starts[a[order_a]]

    deg = np.bincount(dst, minlength=n0)
    e_order = np.argsort(dst, kind="stable")
    vstart = np.zeros(n0 + 1, np.int64)
    np.cumsum(deg, out=vstart[1:])

    vorder = np.argsort(-deg, kind="stable")
    nrows = (n0 + NB - 1) // NB
    pad_n = nrows * NB
    vpad = np.full(pad_n, -1, np.int64)
    vpad[:n0] = vorder
    grid = vpad.reshape(nrows, NB)
    grid[1::2] = grid[1::2, ::-1]
    buckets = [grid[:, b][grid[:, b] >= 0] for b in range(NB)]

    nv_eq = max(len(b) for b in buckets)
    nv_eq = ((nv_eq + 15) // 16) * 16

    maxdeg = int(deg.max()) if n0 else 0
    n_k = []
    for k in range(maxdeg):
        w = max(int((deg[b] > k).sum()) for b in buckets)
        if w == 0:
            break
        n_k.append(w)
    n_k[0] = nv_eq
    offs = np.zeros(len(n_k) + 1, np.int64)
    np.cumsum(n_k, out=offs[1:])
    L = int(offs[-1])
    Lp = ((L + 255) // 256) * 256

    s_idx_e = (t * NA + a[src]).astype(np.int64)
    c2_idx_e = (a[dst] * R + t).astype(np.int64)
    rank_src_e = rank[src].astype(np.float32)

    def wrap(x, width):
        return (x.reshape(G, width // 16, 16)
                 .transpose(0, 2, 1)
                 .reshape(G * 16, width // 16))

    in_maps = []
    for n in range(NCORES):
        sIdx = np.full((G, Lp), DEAD_S, np.int64)
        c2Idx = np.full((G, Lp), DEAD_C2, np.int64)
        rankS = np.full((G, Lp), 1e9, np.float32)
        aDst = np.full((G, nv_eq), NA, np.int64)
        rankD = np.full((G, nv_eq), 1e9, np.float32)
        for g in range(G):
            bl = buckets[n * G + g]
            nb = len(bl)
            aDst[g, :nb] = a[bl]
            rankD[g, :nb] = rank[bl]
            dg = deg[bl]
            for k in range(len(n_k)):
                rsel = np.nonzero(dg > k)[0]
                if len(rsel) == 0:
                    continue
                eids = e_order[vstart[bl[rsel]] + k]
                cols = offs[k] + rsel
                sIdx[g, cols] = s_idx_e[eids]
                c2Idx[g, cols] = c2_idx_e[eids]
                rankS[g, cols] = rank_src_e[eids]

        m = {
            "sIdxW": np.ascontiguousarray(wrap(sIdx, Lp).astype(np.int16)),
            "c2IdxW": np.ascontiguousarray(wrap(c2Idx, Lp).astype(np.int16)),
            "rankB": np.ascontiguousarray(
                np.repeat(rankS, 16, axis=0).astype(np.float32)),
            "aDstW": np.ascontiguousarray(wrap(aDst, nv_eq).astype(np.int16)),
            "rankDstB": np.ascontiguousarray(
                np.repeat(rankD, 16, axis=0).astype(np.float32)),
        }
        in_maps.append(m)

    meta = {"n_k": n_k, "offs": offs, "L": L, "Lp": Lp, "nv_eq": nv_eq,
            "cnt": cnt}
    return in_maps, meta


def pack_model_inputs(inp, cnt):
    f = lambda x: np.ascontiguousarray(np.asarray(x, np.float32))
    emb = f(inp["emb"])
    sh = {
        "embT": np.ascontiguousarray(emb.T),
        "pw0": f(inp["pw0"]).reshape(16, 1),
        "pw1": f(inp["pw1"]).reshape(16, 1),
        "cntRep": np.ascontiguousarray(np.tile(cnt.astype(np.float32), (NA, 1))),
        "cntCol": cnt.astype(np.float32).reshape(NA, 1),
        "l1w": f(inp["l1w"]), "l1b": f(inp["l1b"]).reshape(1, 16),
        "l2w": f(inp["l2w"]), "l2b": f(inp["l2b"]).reshape(1, 4),
        "l3w": f(inp["l3w"]), "l3b": f(inp["l3b"]).reshape(1, 1),
    }
    for l in range(3):
        W = f(inp[f"W{l}"])
        sh[f"Wst{l}"] = np.ascontiguousarray(
            W.transpose(1, 0, 2).reshape(16, R * 16))
        sh[f"qv{l}"] = f(inp[f"q{l}"]).reshape(16, 1)
        sh[f"kv{l}"] = f(inp[f"k{l}"]).reshape(16, 1)
        sh[f"bv{l}"] = f(inp[f"b{l}"]).reshape(16, 1)
    return sh


def seg_matmul_list(meta, n_chunks):
    n_k, offs, Lp = meta["n_k"], meta["offs"], meta["Lp"]
    Wc = Lp // n_chunks
    out = []
    for k, nk in enumerate(n_k):
        lo, hi = int(offs[k]), int(offs[k] + nk)
        p = lo
        pieces = []
        while p < hi:
            q = min(hi, (p // Wc + 1) * Wc)
            r0, r1 = p - lo, q - lo
            if r0 < 512 < r1:
                pieces += [(p, lo + 512), (lo + 512, q)]
            else:
                pieces.append((p, q))
            p = q
        for (p0, p1) in pieces:
            out.append({
                "chunk": p0 // Wc,
                "rhs_lo": p0 - (p0 // Wc) * Wc,
                "rhs_hi": p1 - (p0 // Wc) * Wc,
                "ps_lo": p0 - lo,
                "start": (k == 0),
            })
    return out


# ------------------------------------------------------------- bass builder

def build_program(meta, n_chunks=16, repeats=1, skip=()):
    n_k, Lp, nv_eq = meta["n_k"], meta["Lp"], meta["nv_eq"]
    Wc = Lp // n_chunks
    assert Wc % 16 == 0
    segs = seg_matmul_list(meta, n_chunks)
    by_chunk = {}
    for e in segs:
        by_chunk.setdefault(e["chunk"], []).append(e)
    NT = NE_TAB

    nc = bacc.Bacc("TRN2", target_bir_lowering=False, debug=False,
                   num_devices=NCORES)

    din = {}
    din["sIdxW"] = nc.dram_tensor("sIdxW", [128, Lp // 16], I16, kind="ExternalInput")
    din["c2IdxW"] = nc.dram_tensor("c2IdxW", [128, Lp // 16], I16, kind="ExternalInput")
    din["rankB"] = nc.dram_tensor("rankB", [128, Lp], F32, kind="ExternalInput")
    din["aDstW"] = nc.dram_tensor("aDstW", [128, nv_eq // 16], I16, kind="ExternalInput")
    din["rankDstB"] = nc.dram_tensor("rankDstB", [128, nv_eq], F32, kind="ExternalInput")
    for name, shape in [("embT", [16, NA]), ("pw0", [16, 1]), ("pw1", [16, 1]),
                        ("cntRep", [NA, NA]), ("cntCol", [NA, 1]),
                        ("l1w", [96, 16]), ("l1b", [1, 16]),
                        ("l2w", [16, 4]), ("l2b", [1, 4]),
                        ("l3w", [4, 1]), ("l3b", [1, 1])]:
        din[name] = nc.dram_tensor(name, shape, F32, kind="ExternalInput")
    for l in range(3):
        din[f"Wst{l}"] = nc.dram_tensor(f"Wst{l}", [16, R * 16], F32, kind="ExternalInput")
        for nm in ("qv", "kv", "bv"):
            din[f"{nm}{l}"] = nc.dram_tensor(f"{nm}{l}", [16, 1], F32, kind="ExternalInput")
    dout = nc.dram_tensor("out", [1], F32, kind="ExternalOutput")

    with tile.TileContext(nc) as tc:
        with (
            tc.tile_pool(name="stat", bufs=1) as stat,
            tc.tile_pool(name="tabp", bufs=1) as tabp,
            tc.tile_pool(name="wrk", bufs=2) as wrk,
            tc.tile_pool(name="tiny", bufs=1) as tiny,
            tc.tile_pool(name="ps_acc", bufs=1, space="PSUM") as ps_acc,
            tc.tile_pool(name="ps_sm", bufs=1, space="PSUM") as ps_sm,
            tc.tile_pool(name="ps_mt", bufs=1, space="PSUM") as ps_mt,
            tc.tile_pool(name="ps_tqk", bufs=2, space="PSUM") as ps_tqk,
            tc.tile_pool(name="dram", bufs=1, space="DRAM") as dram,
        ):
            # ---------- static loads ----------
            sIdx = stat.tile([128, Lp // 16], I16)
            c2Idx = stat.tile([128, Lp // 16], I16)
            rankB = stat.tile([128, Lp], F32)
            aDstW = stat.tile([128, nv_eq // 16], I16)
            rankDstB = stat.tile([128, nv_eq], F32)
            embT = stat.tile([16, NA], F32)
            cntRep = stat.tile([NA, NA], F32)
            cntCol = stat.tile([NA, 1], F32)
            l1w = stat.tile([96, 16], F32)
            l1b = stat.tile([1, 16], F32)
            l2w = stat.tile([16, 4], F32)
            l2b = stat.tile([1, 4], F32)
            l3w = stat.tile([4, 1], F32)
            l3b = stat.tile([1, 1], F32)
            pw = [stat.tile([16, 1], F32, tag=f"pw{i}", name=f"pw{i}") for i in range(2)]
            Wst = [stat.tile([16, R * 16], F32, tag=f"wst{i}", name=f"wst{i}") for i in range(3)]
            qv = [stat.tile([16, 1], F32, tag=f"qv{i}", name=f"qvt{i}") for i in range(3)]
            kv = [stat.tile([16, 1], F32, tag=f"kv{i}", name=f"kvt{i}") for i in range(3)]
            for tl, name in ([(sIdx, "sIdxW"), (c2Idx, "c2IdxW"), (rankB, "rankB"),
                              (aDstW, "aDstW"), (rankDstB, "rankDstB"),
                              (embT, "embT"), (cntRep, "cntRep"), (cntCol, "cntCol"),
                              (l1w, "l1w"), (l1b, "l1b"), (l2w, "l2w"), (l2b, "l2b"),
                              (l3w, "l3w"), (l3b, "l3b"),
                              (pw[0], "pw0"), (pw[1], "pw1")]
                             + [(Wst[l], f"Wst{l}") for l in range(3)]
                             + [(qv[l], f"qv{l}") for l in range(3)]
                             + [(kv[l], f"kv{l}") for l in range(3)]):
                nc.sync.dma_start(out=tl[:], in_=din[name].ap())
            bRep = [stat.tile([128, 1], F32, tag=f"bR{i}", name=f"bR{i}") for i in range(3)]
            for l in range(3):
                nc.sync.dma_start(out=bRep[l][:],
                                  in_=bass.AP(din[f"bv{l}"], 0, [[0, 8], [1, 16]]))

            ident = stat.tile([128, 128], F32)
            make_identity(nc, ident[:])
            ones1 = stat.tile([1, 128], F32)
            nc.vector.memset(ones1[:], 1.0)

            # ---------- prolog: s0, s1, keep counts ----------
            def ps_small(shape, tg="tps"):
                return ps_sm.tile(shape, F32, space="PSUM", tag=tg, name="pstiny")

            def rnorm_of(pwt):
                nrm = ps_small([1, 1])
                nc.tensor.matmul(out=nrm[:], lhsT=pwt[:], rhs=pwt[:],
                                 start=True, stop=True)
                sq = tiny.tile([1, 1], F32, tag="sq")
                nc.scalar.activation(out=sq[:], in_=nrm[:], func=AF.Sqrt)
                rn = tiny.tile([1, 1], F32, tag="rn")
                nc.vector.reciprocal(out=rn[:], in_=sq[:])
                rrep = ps_small([NA, 1])
                nc.tensor.matmul(out=rrep[:], lhsT=ones1[:, :NA], rhs=rn[:],
                                 start=True, stop=True)
                rs = tiny.tile([NA, 1], F32, tag="rs10")
                nc.vector.tensor_copy(out=rs[:], in_=rrep[:])
                return rs

            rn0 = rnorm_of(pw[0])
            s0 = stat.tile([NA, 1], F32)
            dot0 = ps_small([NA, 1])
            nc.tensor.matmul(out=dot0[:], lhsT=embT[:], rhs=pw[0][:],
                             start=True, stop=True)
            nc.scalar.activation(out=s0[:], in_=dot0[:], func=AF.Tanh, scale=rn0[:])
            rn1 = rnorm_of(pw[1])
            dot1 = ps_small([NA, 1])
            nc.tensor.matmul(out=dot1[:], lhsT=embT[:], rhs=pw[1][:],
                             start=True, stop=True)
            d1s = tiny.tile([NA, 1], F32, tag="d1s")
            nc.vector.tensor_tensor(out=d1s[:], in0=dot1[:], in1=s0[:], op=OP.mult)
            s1 = stat.tile([NA, 1], F32)
            nc.scalar.activation(out=s1[:], in_=d1s[:], func=AF.Tanh, scale=rn1[:])

            cc = [stat.tile([NA, 1], F32, tag=f"cc{i}", name=f"cct{i}") for i in range(3)]
            nc.vector.memset(cc[0][:], 1.0)
            nc.vector.tensor_copy(out=cc[1][:], in_=s0[:])
            nc.vector.tensor_tensor(out=cc[2][:], in0=s0[:], in1=s1[:], op=OP.mult)

            def colbc(col_ap, n, m):
                # [n,1] -> [n,m] free-dim broadcast
                return bass.AP(col_ap.tensor, col_ap.offset,
                               [list(col_ap.ap[0]), [0, m]])

            def keep_counts(score_col, quota, prev_col, kk):
                srow_ps = ps_small([1, NA])
                nc.tensor.transpose(out=srow_ps[:], in_=score_col[:],
                                    identity=ident[:NA, :NA])
                srow = tiny.tile([1, NA], F32, tag="srow")
                nc.vector.tensor_copy(out=srow[:], in_=srow_ps[:])
                srep_ps = ps_small([NA, NA])
                nc.tensor.matmul(out=srep_ps[:], lhsT=ones1[:, :NA], rhs=srow[:],
                                 start=True, stop=True)
                gt = tiny.tile([NA, NA], F32, tag="gt")
                nc.vector.tensor_tensor(out=gt[:], in0=srep_ps[:],
                                        in1=colbc(score_col[:], NA, NA), op=OP.is_gt)
                nc.vector.tensor_tensor(out=gt[:], in0=gt[:], in1=cntRep[:], op=OP.mult)
                cum = tiny.tile([NA, 1], F32, tag="cum")
                nc.vector.tensor_reduce(cum[:], gt[:], AX.X, OP.add)
                nc.vector.tensor_scalar(out=kk[:], in0=cum[:], scalar1=-1.0,
                                        scalar2=float(quota), op0=OP.mult, op1=OP.add)
                nc.vector.tensor_scalar(out=kk[:], in0=kk[:], scalar1=0.0,
                                        scalar2=None, op0=OP.max)
                nc.vector.tensor_tensor(out=kk[:], in0=kk[:], in1=prev_col[:], op=OP.min)

            kcol = [cntCol,
                    stat.tile([NA, 1], F32, tag="k1", name="k1"),
                    stat.tile([NA, 1], F32, tag="k2", name="k2")]
            keep_counts(s0, N1, kcol[0], kcol[1])
            keep_counts(s1, N2, kcol[1], kcol[2])

            kTab = []
            for l in range(3):
                kr = stat.tile([1, 16], F32, tag=f"kr{l}", name=f"kr{l}")
                nc.vector.memset(kr[:], 0.0)
                kr_ps = ps_small([1, NA])
                nc.tensor.transpose(out=kr_ps[:], in_=kcol[l][:],
                                    identity=ident[:NA, :NA])
                nc.vector.tensor_copy(out=kr[:, :NA], in_=kr_ps[:])
                kt_ps = ps_small([128, 16])
                nc.tensor.matmul(out=kt_ps[:], lhsT=ones1[:], rhs=kr[:],
                                 start=True, stop=True)
                kt = stat.tile([128, 16], F32, tag=f"kt{l}", name=f"ktt{l}")
                nc.vector.tensor_copy(out=kt[:], in_=kt_ps[:])
                kTab.append(kt)

            partials = stat.tile([128, 8], F32)
            nc.vector.memset(partials[:], 0.0)

            # ---------------- layers ----------------
            for _rep in range(repeats):
                for l in range(3):
                    # embl^T [16, 10]
                    crow_ps = ps_small([1, NA])
                    nc.tensor.transpose(out=crow_ps[:], in_=cc[l][:],
                                        identity=ident[:NA, :NA])
                    crow = tiny.tile([1, NA], F32, tag="crow")
                    nc.vector.tensor_copy(out=crow[:], in_=crow_ps[:])
                    crep_ps = ps_small([16, NA])
                    nc.tensor.matmul(out=crep_ps[:], lhsT=ones1[:, :16],
                                     rhs=crow[:], start=True, stop=True)
                    emblT = tiny.tile([16, NA], F32, tag="emblT")
                    nc.vector.tensor_tensor(out=emblT[:], in0=embT[:],
                                            in1=crep_ps[:], op=OP.mult)

                    # qv/kv replicated to [16, 16] for 16-row Tq/Tk
                    qvR = tiny.tile([16, 16], F32, tag="qvR")
                    kvR = tiny.tile([16, 16], F32, tag="kvR")
                    nc.vector.tensor_copy(out=qvR[:], in_=colbc(qv[l][:], 16, 16))
                    nc.vector.tensor_copy(out=kvR[:], in_=colbc(kv[l][:], 16, 16))

                    # Mt [16, 1140] (s = t*10 + a)
                    MtS = tabp.tile([16, 1140], F32, tag="mts")
                    for blk in range(3):
                        mt_ps = ps_mt.tile([16, 512], F32, space="PSUM", tag="mt")
                        t0 = blk * 38
                        t1 = min(R, t0 + 38)
                        for ti in range(t0, t1):
                            col = (ti - t0) * NA
                            nc.tensor.matmul(
                                out=mt_ps[:, col:col + NA],
                                lhsT=Wst[l][:, ti * 16:(ti + 1) * 16],
                                rhs=emblT[:],
                                start=True, stop=True, skip_group_check=True)
                        cw = (t1 - t0) * NA
                        nc.vector.tensor_copy(out=MtS[:, t0 * NA:t0 * NA + cw],
                                              in_=mt_ps[:, :cw])

                    # staging tables
                    stageS = tabp.tile([16, NT * 4], F32, tag="stgS")
                    stageC = tabp.tile([16, NT * 2], F32, tag="stgC")
                    nc.vector.memset(stageS[:, 1140 * 4:], 0.0)
                    nc.vector.memset(stageC[:, 1140 * 2:], 0.0)
                    sS4 = stageS[:].rearrange("p (s d) -> p s d", d=4)
                    sC2 = stageC[:].rearrange("p (s d) -> p s d", d=2)
                    TqR16 = tabp.tile([16, 1140], F32, tag="tqr16")
                    for (c0, c1) in TQ3:
                        tkp = ps_tqk.tile([16, 512], F32, space="PSUM", tag="tqk")
                        nc.tensor.matmul(out=tkp[:, :c1 - c0], lhsT=kvR[:],
                                         rhs=MtS[:, c0:c1], start=True, stop=True,
                                         skip_group_check=True)
                        nc.scalar.activation(out=sS4[:, c0:c1, 1],
                                             in_=tkp[:, :c1 - c0], func=AF.Exp)
                        nc.scalar.activation(out=sS4[:, c0:c1, 2],
                                             in_=tkp[:, :c1 - c0], func=AF.Exp,
                                             scale=0.2)
                        tqp = ps_tqk.tile([16, 512], F32, space="PSUM", tag="tqk")
                        nc.tensor.matmul(out=tqp[:, :c1 - c0], lhsT=qvR[:],
                                         rhs=MtS[:, c0:c1], start=True, stop=True,
                                         skip_group_check=True)
                        nc.vector.tensor_copy(out=TqR16[:, c0:c1],
                                              in_=tqp[:, :c1 - c0])
                    # Mt plane + thr plane
                    nc.vector.tensor_copy(out=sS4[:, :1140, 0], in_=MtS[:])
                    ktp = kTab[l][:16, :]
                    nc.vector.tensor_copy(
                        out=sS4[:, :1140, 3],
                        in_=bass.AP(ktp.tensor, ktp.offset,
                                    [list(ktp.ap[0]), [0, R], [1, NA]]))
                    # EAq planes from permuted Tq (c2 = a*114 + t <- s' = t*10 + a)
                    tq_perm = bass.AP(TqR16[:].tensor, TqR16[:].offset,
                                      [list(TqR16[:].ap[0]), [1, NA], [NA, R]])
                    nc.scalar.activation(out=sC2[:, :1140, 0], in_=tq_perm,
                                         func=AF.Exp)
                    nc.scalar.activation(out=sC2[:, :1140, 1], in_=tq_perm,
                                         func=AF.Exp, scale=0.2)

                    stable = tabp.tile([128, NT * 4], F32, tag="stable")
                    c2table = tabp.tile([128, NT * 2], F32, tag="c2table")
                    for g in range(G):
                        nc.sync.dma_start(out=stable[16 * g:16 * (g + 1), :],
                                          in_=stageS[:])
                        nc.sync.dma_start(out=c2table[16 * g:16 * (g + 1), :],
                                          in_=stageC[:])

                    # PSUM accumulators
                    w1 = min(512, nv_eq)
                    w2 = nv_eq - w1
                    psU = [ps_acc.tile([128, w1], F32, space="PSUM", tag="psU0", name="psU0")]
                    psS = [ps_acc.tile([128, w1], F32, space="PSUM", tag="psS0", name="psS0")]
                    if w2 > 0:
                        psU.append(ps_acc.tile([128, w2], F32, space="PSUM", tag="psU1", name="psU1"))
                        psS.append(ps_acc.tile([128, w2], F32, space="PSUM", tag="psS1", name="psS1"))

                    # stop flag: last seg entry per psum tile; start flag:
                    # ONLY the first matmul touching a bank (start zeroes the
                    # whole 2KB bank, so later k=0 pieces must not re-start)
                    last_e = {}
                    first_e = {}
                    for ci in range(n_chunks):
                        for e in by_chunk.get(ci, []):
                            tx = 0 if e["ps_lo"] < 512 else 1
                            last_e[tx] = id(e)
                            if tx not in first_e:
                                first_e[tx] = id(e)

                    # main stream
                    for ci in range(n_chunks):
                        i0 = ci * (Wc // 16)
                        sO = wrk.tile([128, Wc * 4], F32, tag="sO")
                        cO = wrk.tile([128, Wc * 2], F32, tag="cO")
                        if "gather" not in skip:
                            nc.gpsimd.ap_gather(
                                out_ap=sO[:].rearrange("p (w d) -> p w d", d=4),
                                in_ap=stable[:].rearrange("p (s d) -> p s d", d=4),
                                idxs_ap=sIdx[:, i0:i0 + Wc // 16],
                                channels=128, num_elems=NT, d=4, num_idxs=Wc)
                            nc.gpsimd.ap_gather(
                                out_ap=cO[:].rearrange("p (w d) -> p w d", d=2),
                                in_ap=c2table[:].rearrange("p (s d) -> p s d", d=2),
                                idxs_ap=c2Idx[:, i0:i0 + Wc // 16],
                                channels=128, num_elems=NT, d=2, num_idxs=Wc)
                        else:
                            nc.vector.memset(sO[:, :128], 1.0)
                            nc.vector.memset(cO[:, :128], 1.0)
                        sv = sO[:].rearrange("p (w d) -> p w d", d=4)
                        cv = cO[:].rearrange("p (w d) -> p w d", d=2)
                        m1 = wrk.tile([128, Wc], F32, tag="m1")
                        m2 = wrk.tile([128, Wc], F32, tag="m2")
                        bt = wrk.tile([128, Wc], F32, tag="bt")
                        if "dve" not in skip:
                            nc.vector.tensor_tensor(out=m1[:], in0=cv[:, :, 0],
                                                    in1=sv[:, :, 1], op=OP.mult)
                            nc.vector.tensor_tensor(out=m2[:], in0=cv[:, :, 1],
                                                    in1=sv[:, :, 2], op=OP.mult)
                            nc.vector.tensor_tensor(out=m1[:], in0=m1[:], in1=m2[:],
                                                    op=OP.max)
                            if l > 0:
                                nc.vector.tensor_tensor(
                                    out=m2[:], in0=rankB[:, ci * Wc:(ci + 1) * Wc],
                                    in1=sv[:, :, 3], op=OP.is_lt)
                                nc.vector.tensor_tensor(out=m1[:], in0=m1[:],
                                                        in1=m2[:], op=OP.mult)
                            nc.vector.tensor_tensor(out=bt[:], in0=m1[:],
                                                    in1=sv[:, :, 0], op=OP.mult)
                        else:
                            nc.vector.memset(m1[:, :128], 1.0)
                            nc.vector.memset(bt[:, :128], 1.0)

                        for e in (by_chunk.get(ci, []) if "pe" not in skip
                                  else [x for x in by_chunk.get(ci, [])
                                        if first_e.get(0 if x["ps_lo"] < 512 else 1) == id(x)
                                        or last_e.get(0 if x["ps_lo"] < 512 else 1) == id(x)]):
                            pl = e["ps_lo"]
                            tix = 0 if pl < 512 else 1
                            pb = pl - tix * 512
                            wdt = e["rhs_hi"] - e["rhs_lo"]
                            is_last = last_e.get(tix) == id(e)
                            is_first = first_e.get(tix) == id(e)
                            nc.tensor.matmul(
                                out=psU[tix][:, pb:pb + wdt], lhsT=ident[:],
                                rhs=bt[:, e["rhs_lo"]:e["rhs_hi"]],
                                start=is_first, stop=is_last,
                                skip_group_check=True)
                            nc.tensor.matmul(
                                out=psS[tix][:, pb:pb + wdt], lhsT=ident[:],
                                rhs=m1[:, e["rhs_lo"]:e["rhs_hi"]],
                                start=is_first, stop=is_last,
                                skip_group_check=True)

                    # per-vertex tail
                    Svec = tiny.tile([128, nv_eq], F32, tag="svec")
                    for tix in range(len(psU)):
                        c0 = tix * 512
                        cw = psS[tix].shape[1]
                        nc.vector.tensor_scalar(out=Svec[:, c0:c0 + cw],
                                                in0=psS[tix][:], scalar1=1e-16,
                                                scalar2=None, op0=OP.add)
                    rS = tiny.tile([128, nv_eq], F32, tag="rsv")
                    nc.vector.reciprocal(out=rS[:], in_=Svec[:])
                    outv = tiny.tile([128, nv_eq], F32, tag="outv")
                    for tix in range(len(psU)):
                        c0 = tix * 512
                        cw = psU[tix].shape[1]
                        nc.vector.tensor_tensor(out=outv[:, c0:c0 + cw],
                                                in0=psU[tix][:],
                                                in1=rS[:, c0:c0 + cw], op=OP.mult)
                    h = tiny.tile([128, nv_eq], F32, tag="h")
                    nc.scalar.activation(out=h[:], in_=outv[:], func=AF.Relu,
                                         bias=bRep[l][:])
                    thrD = tiny.tile([128, nv_eq], F32, tag="thrd")
                    nc.gpsimd.ap_gather(
                        out_ap=thrD[:].rearrange("p (w d) -> p w d", d=1),
                        in_ap=kTab[l][:].rearrange("p (s d) -> p s d", d=1),
                        idxs_ap=aDstW[:], channels=128, num_elems=16, d=1,
                        num_idxs=nv_eq)
                    alive = tiny.tile([128, nv_eq], F32, tag="alive")
                    nc.vector.tensor_tensor(out=alive[:], in0=rankDstB[:],
                                            in1=thrD[:], op=OP.is_lt)
                    nc.vector.tensor_tensor(out=h[:], in0=h[:], in1=alive[:],
                                            op=OP.mult)
                    nc.vector.tensor_reduce(partials[:, l:l + 1], h[:], AX.X, OP.add)
                    nc.vector.tensor_reduce(partials[:, 3 + l:4 + l], h[:], AX.X, OP.max)

                # ---------------- combine + MLP ----------------
                cc_in = dram.tile([128, 8], F32)
                cc_out = dram.tile([NCORES * 128, 8], F32)
                nc.sync.dma_start(out=cc_in[:], in_=partials[:])
                nc.gpsimd.collective_compute(
                    "AllGather", OP.bypass,
                    replica_groups=[list(range(NCORES))],
                    ins=[cc_in[:].opt()], outs=[cc_out[:].opt()])
                allp = tiny.tile([128, NCORES * 8], F32, tag="allp")
                nc.sync.dma_start(
                    out=allp[:],
                    in_=bass.AP(cc_out[:].tensor, cc_out[:].offset,
                                [[8, 128], [1024, NCORES], [1, 8]]))
                comb = tiny.tile([128, 8], F32, tag="comb")
                nc.vector.memset(comb[:], 0.0)
                ab = allp[:]
                nc.vector.tensor_reduce(
                    comb[:, 0:3],
                    bass.AP(ab.tensor, ab.offset,
                            [list(ab.ap[0]), [1, 3], [8, NCORES]]),
                    AX.X, OP.add)
                nc.vector.tensor_reduce(
                    comb[:, 3:6],
                    bass.AP(ab.tensor, ab.offset + 3,
                            [list(ab.ap[0]), [1, 3], [8, NCORES]]),
                    AX.X, OP.max)
                shf = tiny.tile([128, 8], F32, tag="shf")
                for sh in (64, 32, 16):
                    nc.sync.dma_start(out=shf[:sh, :], in_=comb[sh:2 * sh, :])
                    nc.vector.tensor_tensor(out=comb[:sh, 0:3], in0=comb[:sh, 0:3],
                                            in1=shf[:sh, 0:3], op=OP.add)
                    nc.vector.tensor_tensor(out=comb[:sh, 3:6], in0=comb[:sh, 3:6],
                                            in1=shf[:sh, 3:6], op=OP.max)
                for l in range(3):
                    nc.vector.tensor_scalar(out=comb[:16, l:l + 1],
                                            in0=comb[:16, l:l + 1],
                                            scalar1=1.0 / QUOTAS[l], scalar2=None,
                                            op0=OP.mult)
                gcol = tiny.tile([96, 1], F32, tag="gcol")
                for l in range(3):
                    nc.sync.dma_start(out=gcol[32 * l:32 * l + 16, :],
                                      in_=comb[:16, l:l + 1])
                    nc.sync.dma_start(out=gcol[32 * l + 16:32 * l + 32, :],
                                      in_=comb[:16, 3 + l:4 + l])
                z1_ps = ps_small([1, 16])
                nc.tensor.matmul(out=z1_ps[:], lhsT=gcol[:], rhs=l1w[:],
                                 start=True, stop=True)
                h1 = tiny.tile([1, 16], F32, tag="h1")
                nc.vector.tensor_tensor(out=h1[:], in0=z1_ps[:], in1=l1b[:], op=OP.add)
                nc.scalar.activation(out=h1[:], in_=h1[:], func=AF.Relu)
                h1c_ps = ps_small([16, 1])
                nc.tensor.transpose(out=h1c_ps[:], in_=h1[:], identity=ident[:1, :1])
                h1c = tiny.tile([16, 1], F32, tag="h1c")
                nc.vector.tensor_copy(out=h1c[:], in_=h1c_ps[:])
                z2_ps = ps_small([1, 4])
                nc.tensor.matmul(out=z2_ps[:], lhsT=h1c[:], rhs=l2w[:],
                                 start=True, stop=True)
                h2 = tiny.tile([1, 4], F32, tag="h2")
                nc.vector.tensor_tensor(out=h2[:], in0=z2_ps[:], in1=l2b[:], op=OP.add)
                nc.scalar.activation(out=h2[:], in_=h2[:], func=AF.Relu)
                h2c_ps = ps_small([4, 1])
                nc.tensor.transpose(out=h2c_ps[:], in_=h2[:], identity=ident[:1, :1])
                h2c = tiny.tile([4, 1], F32, tag="h2c")
                nc.vector.tensor_copy(out=h2c[:], in_=h2c_ps[:])
                z3_ps = ps_small([1, 1])
                nc.tensor.matmul(out=z3_ps[:], lhsT=h2c[:], rhs=l3w[:],
                                 start=True, stop=True)
                z3 = tiny.tile([1, 1], F32, tag="z3")
                nc.vector.tensor_tensor(out=z3[:], in0=z3_ps[:], in1=l3b[:], op=OP.add)
                sig = tiny.tile([1, 1], F32, tag="sig")
                nc.scalar.activation(out=sig[:], in_=z3[:], func=AF.Sigmoid)
                nc.sync.dma_start(out=dout.ap(), in_=sig[:])

    nc.finalize()
    return nc


# ------------------------------------------------------------------ driver

_CACHE = {}


def kernel(**inputs):
    in_maps_nc, meta = host_prep(inputs["node_ids"], inputs["edge_index"],
                                 inputs["edge_type"])
    shared = pack_model_inputs(inputs, meta["cnt"])
    in_maps = [{**m, **shared} for m in in_maps_nc]

    key = (meta["Lp"], meta["nv_eq"], tuple(meta["n_k"]))
    if key not in _CACHE:
        _CACHE[key] = build_program(meta)
    nc = _CACHE[key]

    res = run_bass_kernel_spmd(nc, in_maps, core_ids=list(range(NCORES)))
    return np.asarray(res.results[0]["out"], np.float32)



# revision 5
# speedup vs baseline: 416.5209x; 416.5209x over previous
"""Trainium2 Bass kernel for nn_DetectModel (RGAT x3 + TopK pooling + MLP head).

Structure exploited (validated vs reference):
  - x_l[v] = emb[a_v] * c_l(a_v): pooling only scales node features by
    attr-dependent tanh scores, so all per-edge quantities come from tiny
    tables indexed by s = t*10 + a_src (1140) and c2 = a_dst*114 + t (1140).
  - P_e = exp(lrelu(Tq[c2] + Tk[s])) = max(EAq[c2]*EAk[s], E2Aq[c2]*E2Ak[s])
    with pre-exponentiated tables; the segment max cancels in the softmax.
  - Node v survives pool l  <=>  rank_a(v) < k_l[a_v]  (k_l computed on
    device from tanh scores + host-known attr counts).
  - Sharding: dst-vertex sharding into 64 degree-balanced buckets
    (8 NeuronCores x 8 GPSIMD core-groups); segment reductions never cross
    cores. ELL layout (k-th in-edge per vertex, vertices sorted by degree)
    turns segment sums into identity-matmul PSUM accumulations.

Device pipeline per layer: GPSIMD ap_gather (tables, per-core-broadcast) ->
DVE elementwise (products, mask) -> PE identity-matmul segment accumulation
into PSUM (U, S) -> per-vertex tail (recip, relu+bias, alive mask, reduce).
Final: one AllGather of [128, 8] partials, replicated tiny MLP, out[1].
"""
import numpy as np

import concourse.bass as bass
import concourse.bacc as bacc
import concourse.mybir as mybir
import concourse.tile as tile
from concourse.bass_utils import run_bass_kernel_spmd
from concourse.masks import make_identity

F32 = mybir.dt.float32
I16 = mybir.dt.int16
AF = mybir.ActivationFunctionType
OP = mybir.AluOpType
AX = mybir.AxisListType

N0, N1, N2 = 50000, 40000, 32000
E = 600000
D = 16
R = 114
NA = 10
NCORES = 8
G = 8
NB = NCORES * G

NE_TAB = 1152          # table slots (>= 1141), dead index = 1140
DEAD_S = 1140
DEAD_C2 = 1140
QUOTAS = (N0, N1, N2)
TQ3 = ((0, 380), (380, 760), (760, 1140))


# ---------------------------------------------------------------- host prep

def host_prep(node_ids, edge_index, edge_type):
    a = np.asarray(node_ids).astype(np.int64)
    src = np.asarray(edge_index)[0].astype(np.int64)
    dst = np.asarray(edge_index)[1].astype(np.int64)
    t = np.asarray(edge_type).astype(np.int64)
    n0 = a.shape[0]

    cnt = np.bincount(a, minlength=NA).astype(np.int64)
    order_a = np.argsort(a, kind="stable")
    rank = np.empty(n0, np.int64)
    starts = np.zeros(NA + 1, np.int64)
    np.cumsum(cnt, out=starts[1:])
    rank[order_a] = np.arange(n0) - starts[a[order_a]]

    deg = np.bincount(dst, minlength=n0)
    e_order = np.argsort(dst, kind="stable")
    vstart = np.zeros(n0 + 1, np.int64)
    np.cumsum(deg, out=vstart[1:])

    vorder = np.argsort(-deg, kind="stable")
    nrows = (n0 + NB - 1) // NB
    pad_n = nrows * NB
    vpad = np.full(pad_n, -1, np.int64)
    vpad[:n0] = vorder
    grid = vpad.reshape(nrows, NB)
    grid[1::2] = grid[1::2, ::-1]
    buckets = [grid[:, b][grid[:, b] >= 0] for b in range(NB)]

    nv_eq = max(len(b) for b in buckets)
    nv_eq = ((nv_eq + 15) // 16) * 16

    maxdeg = int(deg.max()) if n0 else 0
    n_k = []
    for k in range(maxdeg):
        w = max(int((deg[b] > k).sum()) for b in buckets)
        if w == 0:
            break
        n_k.append(w)
    n_k[0] = nv_eq
    offs = np.zeros(len(n_k) + 1, np.int64)
    np.cumsum(n_k, out=offs[1:])
    L = int(offs[-1])
    Lp = ((L + 255) // 256) * 256

    s_idx_e = (t * NA + a[src]).astype(np.int64)
    c2_idx_e = (a[dst] * R + t).astype(np.int64)
    rank_src_e = rank[src].astype(np.float32)

    def wrap(x, width):
        return (x.reshape(G, width // 16, 16)
                 .transpose(0, 2, 1)
                 .reshape(G * 16, width // 16))

    in_maps = []
    for n in range(NCORES):
        sIdx = np.full((G, Lp), DEAD_S, np.int64)
        c2Idx = np.full((G, Lp), DEAD_C2, np.int64)
        rankS = np.full((G, Lp), 1e9, np.float32)
        aDst = np.full((G, nv_eq), NA, np.int64)
        rankD = np.full((G, nv_eq), 1e9, np.float32)
        for g in range(G):
            bl = buckets[n * G + g]
            nb = len(bl)
            aDst[g, :nb] = a[bl]
            rankD[g, :nb] = rank[bl]
            dg = deg[bl]
            for k in range(len(n_k)):
                rsel = np.nonzero(dg > k)[0]
                if len(rsel) == 0:
                    continue
                eids = e_order[vstart[bl[rsel]] + k]
                cols = offs[k] + rsel
                sIdx[g, cols] = s_idx_e[eids]
                c2Idx[g, cols] = c2_idx_e[eids]
                rankS[g, cols] = rank_src_e[eids]

        m = {
            "sIdxW": np.ascontiguousarray(wrap(sIdx, Lp).astype(np.int16)),
            "c2IdxW": np.ascontiguousarray(wrap(c2Idx, Lp).astype(np.int16)),
            "rankB": np.ascontiguousarray(
                np.repeat(rankS, 16, axis=0).astype(np.float32)),
            "aDstW": np.ascontiguousarray(wrap(aDst, nv_eq).astype(np.int16)),
            "rankDstB": np.ascontiguousarray(
                np.repeat(rankD, 16, axis=0).astype(np.float32)),
        }
        in_maps.append(m)

    meta = {"n_k": n_k, "offs": offs, "L": L, "Lp": Lp, "nv_eq": nv_eq,
            "cnt": cnt}
    return in_maps, meta


def pack_model_inputs(inp, cnt):
    f = lambda x: np.ascontiguousarray(np.asarray(x, np.float32))
    emb = f(inp["emb"])
    sh = {
        "embT": np.ascontiguousarray(emb.T),
        "pw0": f(inp["pw0"]).reshape(16, 1),
        "pw1": f(inp["pw1"]).reshape(16, 1),
        "cntRep": np.ascontiguousarray(np.tile(cnt.astype(np.float32), (NA, 1))),
        "cntCol": cnt.astype(np.float32).reshape(NA, 1),
        "l1w": f(inp["l1w"]), "l1b": f(inp["l1b"]).reshape(1, 16),
        "l2w": f(inp["l2w"]), "l2b": f(inp["l2b"]).reshape(1, 4),
        "l3w": f(inp["l3w"]), "l3b": f(inp["l3b"]).reshape(1, 1),
    }
    for l in range(3):
        W = f(inp[f"W{l}"])
        sh[f"Wst{l}"] = np.ascontiguousarray(
            W.transpose(1, 0, 2).reshape(16, R * 16))
        sh[f"qv{l}"] = f(inp[f"q{l}"]).reshape(16, 1)
        sh[f"kv{l}"] = f(inp[f"k{l}"]).reshape(16, 1)
        sh[f"bv{l}"] = f(inp[f"b{l}"]).reshape(16, 1)
    return sh


def seg_matmul_list(meta, n_chunks):
    n_k, offs, Lp = meta["n_k"], meta["offs"], meta["Lp"]
    Wc = Lp // n_chunks
    out = []
    for k, nk in enumerate(n_k):
        lo, hi = int(offs[k]), int(offs[k] + nk)
        p = lo
        pieces = []
        while p < hi:
            q = min(hi, (p // Wc + 1) * Wc)
            r0, r1 = p - lo, q - lo
            if r0 < 512 < r1:
                pieces += [(p, lo + 512), (lo + 512, q)]
            else:
                pieces.append((p, q))
            p = q
        for (p0, p1) in pieces:
            out.append({
                "chunk": p0 // Wc,
                "rhs_lo": p0 - (p0 // Wc) * Wc,
                "rhs_hi": p1 - (p0 // Wc) * Wc,
                "ps_lo": p0 - lo,
                "start": (k == 0),
            })
    return out


# ------------------------------------------------------------- bass builder

def build_program(meta, n_chunks=16, repeats=1, skip=(), num_devices=NCORES,
                  tail=True):
    n_k, Lp, nv_eq = meta["n_k"], meta["Lp"], meta["nv_eq"]
    Wc = Lp // n_chunks
    assert Wc % 16 == 0
    segs = seg_matmul_list(meta, n_chunks)
    by_chunk = {}
    for e in segs:
        by_chunk.setdefault(e["chunk"], []).append(e)
    NT = NE_TAB

    nc = bacc.Bacc("TRN2", target_bir_lowering=False, debug=False,
                   num_devices=num_devices)

    din = {}
    din["sIdxW"] = nc.dram_tensor("sIdxW", [128, Lp // 16], I16, kind="ExternalInput")
    din["c2IdxW"] = nc.dram_tensor("c2IdxW", [128, Lp // 16], I16, kind="ExternalInput")
    din["rankB"] = nc.dram_tensor("rankB", [128, Lp], F32, kind="ExternalInput")
    din["aDstW"] = nc.dram_tensor("aDstW", [128, nv_eq // 16], I16, kind="ExternalInput")
    din["rankDstB"] = nc.dram_tensor("rankDstB", [128, nv_eq], F32, kind="ExternalInput")
    for name, shape in [("embT", [16, NA]), ("pw0", [16, 1]), ("pw1", [16, 1]),
                        ("cntRep", [NA, NA]), ("cntCol", [NA, 1]),
                        ("l1w", [96, 16]), ("l1b", [1, 16]),
                        ("l2w", [16, 4]), ("l2b", [1, 4]),
                        ("l3w", [4, 1]), ("l3b", [1, 1])]:
        din[name] = nc.dram_tensor(name, shape, F32, kind="ExternalInput")
    for l in range(3):
        din[f"Wst{l}"] = nc.dram_tensor(f"Wst{l}", [16, R * 16], F32, kind="ExternalInput")
        for nm in ("qv", "kv", "bv"):
            din[f"{nm}{l}"] = nc.dram_tensor(f"{nm}{l}", [16, 1], F32, kind="ExternalInput")
    dout = nc.dram_tensor("out", [1], F32, kind="ExternalOutput")

    with tile.TileContext(nc) as tc:
        with (
            tc.tile_pool(name="stat", bufs=1) as stat,
            tc.tile_pool(name="tabp", bufs=1) as tabp,
            tc.tile_pool(name="wrk", bufs=2) as wrk,
            tc.tile_pool(name="tiny", bufs=1) as tiny,
            tc.tile_pool(name="ps_acc", bufs=1, space="PSUM") as ps_acc,
            tc.tile_pool(name="ps_sm", bufs=1, space="PSUM") as ps_sm,
            tc.tile_pool(name="ps_mt", bufs=1, space="PSUM") as ps_mt,
            tc.tile_pool(name="ps_tqk", bufs=2, space="PSUM") as ps_tqk,
            tc.tile_pool(name="dram", bufs=1, space="DRAM") as dram,
        ):
            # ---------- static loads ----------
            sIdx = stat.tile([128, Lp // 16], I16)
            c2Idx = stat.tile([128, Lp // 16], I16)
            rankB = stat.tile([128, Lp], F32)
            aDstW = stat.tile([128, nv_eq // 16], I16)
            rankDstB = stat.tile([128, nv_eq], F32)
            embT = stat.tile([16, NA], F32)
            cntRep = stat.tile([NA, NA], F32)
            cntCol = stat.tile([NA, 1], F32)
            l1w = stat.tile([96, 16], F32)
            l1b = stat.tile([1, 16], F32)
            l2w = stat.tile([16, 4], F32)
            l2b = stat.tile([1, 4], F32)
            l3w = stat.tile([4, 1], F32)
            l3b = stat.tile([1, 1], F32)
            pw = [stat.tile([16, 1], F32, tag=f"pw{i}", name=f"pw{i}") for i in range(2)]
            Wst = [stat.tile([16, R * 16], F32, tag=f"wst{i}", name=f"wst{i}") for i in range(3)]
            qv = [stat.tile([16, 1], F32, tag=f"qv{i}", name=f"qvt{i}") for i in range(3)]
            kv = [stat.tile([16, 1], F32, tag=f"kv{i}", name=f"kvt{i}") for i in range(3)]
            for tl, name in ([(sIdx, "sIdxW"), (c2Idx, "c2IdxW"), (rankB, "rankB"),
                              (aDstW, "aDstW"), (rankDstB, "rankDstB"),
                              (embT, "embT"), (cntRep, "cntRep"), (cntCol, "cntCol"),
                              (l1w, "l1w"), (l1b, "l1b"), (l2w, "l2w"), (l2b, "l2b"),
                              (l3w, "l3w"), (l3b, "l3b"),
                              (pw[0], "pw0"), (pw[1], "pw1")]
                             + [(Wst[l], f"Wst{l}") for l in range(3)]
                             + [(qv[l], f"qv{l}") for l in range(3)]
                             + [(kv[l], f"kv{l}") for l in range(3)]):
                nc.sync.dma_start(out=tl[:], in_=din[name].ap())
            bRep = [stat.tile([128, 1], F32, tag=f"bR{i}", name=f"bR{i}") for i in range(3)]
            for l in range(3):
                nc.sync.dma_start(out=bRep[l][:],
                                  in_=bass.AP(din[f"bv{l}"], 0, [[0, 8], [1, 16]]))

            ident = stat.tile([128, 128], F32)
            make_identity(nc, ident[:])
            ones1 = stat.tile([1, 128], F32)
            nc.vector.memset(ones1[:], 1.0)

            # ---------- prolog: s0, s1, keep counts ----------
            def ps_small(shape, tg="tps"):
                return ps_sm.tile(shape, F32, space="PSUM", tag=tg, name="pstiny")

            def rnorm_of(pwt):
                nrm = ps_small([1, 1])
                nc.tensor.matmul(out=nrm[:], lhsT=pwt[:], rhs=pwt[:],
                                 start=True, stop=True)
                sq = tiny.tile([1, 1], F32, tag="sq")
                nc.scalar.activation(out=sq[:], in_=nrm[:], func=AF.Sqrt)
                rn = tiny.tile([1, 1], F32, tag="rn")
                nc.vector.reciprocal(out=rn[:], in_=sq[:])
                rrep = ps_small([NA, 1])
                nc.tensor.matmul(out=rrep[:], lhsT=ones1[:, :NA], rhs=rn[:],
                                 start=True, stop=True)
                rs = tiny.tile([NA, 1], F32, tag="rs10")
                nc.vector.tensor_copy(out=rs[:], in_=rrep[:])
                return rs

            rn0 = rnorm_of(pw[0])
            s0 = stat.tile([NA, 1], F32)
            dot0 = ps_small([NA, 1])
            nc.tensor.matmul(out=dot0[:], lhsT=embT[:], rhs=pw[0][:],
                             start=True, stop=True)
            nc.scalar.activation(out=s0[:], in_=dot0[:], func=AF.Tanh, scale=rn0[:])
            rn1 = rnorm_of(pw[1])
            dot1 = ps_small([NA, 1])
            nc.tensor.matmul(out=dot1[:], lhsT=embT[:], rhs=pw[1][:],
                             start=True, stop=True)
            d1s = tiny.tile([NA, 1], F32, tag="d1s")
            nc.vector.tensor_tensor(out=d1s[:], in0=dot1[:], in1=s0[:], op=OP.mult)
            s1 = stat.tile([NA, 1], F32)
            nc.scalar.activation(out=s1[:], in_=d1s[:], func=AF.Tanh, scale=rn1[:])

            cc = [stat.tile([NA, 1], F32, tag=f"cc{i}", name=f"cct{i}") for i in range(3)]
            nc.vector.memset(cc[0][:], 1.0)
            nc.vector.tensor_copy(out=cc[1][:], in_=s0[:])
            nc.vector.tensor_tensor(out=cc[2][:], in0=s0[:], in1=s1[:], op=OP.mult)

            def colbc(col_ap, n, m):
                # [n,1] -> [n,m] free-dim broadcast
                return bass.AP(col_ap.tensor, col_ap.offset,
                               [list(col_ap.ap[0]), [0, m]])

            def keep_counts(score_col, quota, prev_col, kk):
                srow_ps = ps_small([1, NA])
                nc.tensor.transpose(out=srow_ps[:], in_=score_col[:],
                                    identity=ident[:NA, :NA])
                srow = tiny.tile([1, NA], F32, tag="srow")
                nc.vector.tensor_copy(out=srow[:], in_=srow_ps[:])
                srep_ps = ps_small([NA, NA])
                nc.tensor.matmul(out=srep_ps[:], lhsT=ones1[:, :NA], rhs=srow[:],
                                 start=True, stop=True)
                gt = tiny.tile([NA, NA], F32, tag="gt")
                nc.vector.tensor_tensor(out=gt[:], in0=srep_ps[:],
                                        in1=colbc(score_col[:], NA, NA), op=OP.is_gt)
                nc.vector.tensor_tensor(out=gt[:], in0=gt[:], in1=cntRep[:], op=OP.mult)
                cum = tiny.tile([NA, 1], F32, tag="cum")
                nc.vector.tensor_reduce(cum[:], gt[:], AX.X, OP.add)
                nc.vector.tensor_scalar(out=kk[:], in0=cum[:], scalar1=-1.0,
                                        scalar2=float(quota), op0=OP.mult, op1=OP.add)
                nc.vector.tensor_scalar(out=kk[:], in0=kk[:], scalar1=0.0,
                                        scalar2=None, op0=OP.max)
                nc.vector.tensor_tensor(out=kk[:], in0=kk[:], in1=prev_col[:], op=OP.min)

            kcol = [cntCol,
                    stat.tile([NA, 1], F32, tag="k1", name="k1"),
                    stat.tile([NA, 1], F32, tag="k2", name="k2")]
            keep_counts(s0, N1, kcol[0], kcol[1])
            keep_counts(s1, N2, kcol[1], kcol[2])

            kTab = []
            for l in range(3):
                kr = stat.tile([1, 16], F32, tag=f"kr{l}", name=f"kr{l}")
                nc.vector.memset(kr[:], 0.0)
                kr_ps = ps_small([1, NA])
                nc.tensor.transpose(out=kr_ps[:], in_=kcol[l][:],
                                    identity=ident[:NA, :NA])
                nc.vector.tensor_copy(out=kr[:, :NA], in_=kr_ps[:])
                kt_ps = ps_small([128, 16])
                nc.tensor.matmul(out=kt_ps[:], lhsT=ones1[:], rhs=kr[:],
                                 start=True, stop=True)
                kt = stat.tile([128, 16], F32, tag=f"kt{l}", name=f"ktt{l}")
                nc.vector.tensor_copy(out=kt[:], in_=kt_ps[:])
                kTab.append(kt)

            partials = stat.tile([128, 8], F32)
            nc.vector.memset(partials[:], 0.0)

            # ---------------- layers ----------------
            for _rep in range(repeats):
                for l in range(3):
                    # embl^T [16, 10]
                    crow_ps = ps_small([1, NA])
                    nc.tensor.transpose(out=crow_ps[:], in_=cc[l][:],
                                        identity=ident[:NA, :NA])
                    crow = tiny.tile([1, NA], F32, tag="crow")
                    nc.vector.tensor_copy(out=crow[:], in_=crow_ps[:])
                    crep_ps = ps_small([16, NA])
                    nc.tensor.matmul(out=crep_ps[:], lhsT=ones1[:, :16],
                                     rhs=crow[:], start=True, stop=True)
                    emblT = tiny.tile([16, NA], F32, tag="emblT")
                    nc.vector.tensor_tensor(out=emblT[:], in0=embT[:],
                                            in1=crep_ps[:], op=OP.mult)

                    # qv/kv replicated to [16, 16] for 16-row Tq/Tk
                    qvR = tiny.tile([16, 16], F32, tag="qvR")
                    kvR = tiny.tile([16, 16], F32, tag="kvR")
                    nc.vector.tensor_copy(out=qvR[:], in_=colbc(qv[l][:], 16, 16))
                    nc.vector.tensor_copy(out=kvR[:], in_=colbc(kv[l][:], 16, 16))

                    # Mt [16, 1140] (s = t*10 + a)
                    MtS = tabp.tile([16, 1140], F32, tag="mts")
                    for blk in range(3):
                        mt_ps = ps_mt.tile([16, 512], F32, space="PSUM", tag="mt")
                        t0 = blk * 38
                        t1 = min(R, t0 + 38)
                        for ti in range(t0, t1):
                            col = (ti - t0) * NA
                            nc.tensor.matmul(
                                out=mt_ps[:, col:col + NA],
                                lhsT=Wst[l][:, ti * 16:(ti + 1) * 16],
                                rhs=emblT[:],
                                start=True, stop=True, skip_group_check=True)
                        cw = (t1 - t0) * NA
                        nc.vector.tensor_copy(out=MtS[:, t0 * NA:t0 * NA + cw],
                                              in_=mt_ps[:, :cw])

                    # staging tables
                    stageS = tabp.tile([16, NT * 4], F32, tag="stgS")
                    stageC = tabp.tile([16, NT * 2], F32, tag="stgC")
                    nc.vector.memset(stageS[:, 1140 * 4:], 0.0)
                    nc.vector.memset(stageC[:, 1140 * 2:], 0.0)
                    sS4 = stageS[:].rearrange("p (s d) -> p s d", d=4)
                    sC2 = stageC[:].rearrange("p (s d) -> p s d", d=2)
                    TqR16 = tabp.tile([16, 1140], F32, tag="tqr16")
                    for (c0, c1) in TQ3:
                        tkp = ps_tqk.tile([16, 512], F32, space="PSUM", tag="tqk")
                        nc.tensor.matmul(out=tkp[:, :c1 - c0], lhsT=kvR[:],
                                         rhs=MtS[:, c0:c1], start=True, stop=True,
                                         skip_group_check=True)
                        nc.scalar.activation(out=sS4[:, c0:c1, 1],
                                             in_=tkp[:, :c1 - c0], func=AF.Exp)
                        nc.scalar.activation(out=sS4[:, c0:c1, 2],
                                             in_=tkp[:, :c1 - c0], func=AF.Exp,
                                             scale=0.2)
                        tqp = ps_tqk.tile([16, 512], F32, space="PSUM", tag="tqk")
                        nc.tensor.matmul(out=tqp[:, :c1 - c0], lhsT=qvR[:],
                                         rhs=MtS[:, c0:c1], start=True, stop=True,
                                         skip_group_check=True)
                        nc.vector.tensor_copy(out=TqR16[:, c0:c1],
                                              in_=tqp[:, :c1 - c0])
                    # Mt plane + thr plane
                    nc.vector.tensor_copy(out=sS4[:, :1140, 0], in_=MtS[:])
                    ktp = kTab[l][:16, :]
                    nc.vector.tensor_copy(
                        out=sS4[:, :1140, 3],
                        in_=bass.AP(ktp.tensor, ktp.offset,
                                    [list(ktp.ap[0]), [0, R], [1, NA]]))
                    # EAq planes from permuted Tq (c2 = a*114 + t <- s' = t*10 + a)
                    tq_perm = bass.AP(TqR16[:].tensor, TqR16[:].offset,
                                      [list(TqR16[:].ap[0]), [1, NA], [NA, R]])
                    nc.scalar.activation(out=sC2[:, :1140, 0], in_=tq_perm,
                                         func=AF.Exp)
                    nc.scalar.activation(out=sC2[:, :1140, 1], in_=tq_perm,
                                         func=AF.Exp, scale=0.2)

                    stable = tabp.tile([128, NT * 4], F32, tag="stable")
                    c2table = tabp.tile([128, NT * 2], F32, tag="c2table")
                    for g in range(G):
                        nc.sync.dma_start(out=stable[16 * g:16 * (g + 1), :],
                                          in_=stageS[:])
                        nc.sync.dma_start(out=c2table[16 * g:16 * (g + 1), :],
                                          in_=stageC[:])

                    # PSUM accumulators
                    w1 = min(512, nv_eq)
                    w2 = nv_eq - w1
                    psU = [ps_acc.tile([128, w1], F32, space="PSUM", tag="psU0", name="psU0")]
                    psS = [ps_acc.tile([128, w1], F32, space="PSUM", tag="psS0", name="psS0")]
                    if w2 > 0:
                        psU.append(ps_acc.tile([128, w2], F32, space="PSUM", tag="psU1", name="psU1"))
                        psS.append(ps_acc.tile([128, w2], F32, space="PSUM", tag="psS1", name="psS1"))

                    # stop flag: last seg entry per psum tile; start flag:
                    # ONLY the first matmul touching a bank (start zeroes the
                    # whole 2KB bank, so later k=0 pieces must not re-start)
                    last_e = {}
                    first_e = {}
                    for ci in range(n_chunks):
                        for e in by_chunk.get(ci, []):
                            tx = 0 if e["ps_lo"] < 512 else 1
                            last_e[tx] = id(e)
                            if tx not in first_e:
                                first_e[tx] = id(e)

                    # main stream
                    for ci in range(n_chunks):
                        i0 = ci * (Wc // 16)
                        sO = wrk.tile([128, Wc * 4], F32, tag="sO")
                        cO = wrk.tile([128, Wc * 2], F32, tag="cO")
                        if "gather" not in skip:
                            nc.gpsimd.ap_gather(
                                out_ap=sO[:].rearrange("p (w d) -> p w d", d=4),
                                in_ap=stable[:].rearrange("p (s d) -> p s d", d=4),
                                idxs_ap=sIdx[:, i0:i0 + Wc // 16],
                                channels=128, num_elems=NT, d=4, num_idxs=Wc)
                            nc.gpsimd.ap_gather(
                                out_ap=cO[:].rearrange("p (w d) -> p w d", d=2),
                                in_ap=c2table[:].rearrange("p (s d) -> p s d", d=2),
                                idxs_ap=c2Idx[:, i0:i0 + Wc // 16],
                                channels=128, num_elems=NT, d=2, num_idxs=Wc)
                        else:
                            nc.vector.memset(sO[:, :128], 1.0)
                            nc.vector.memset(cO[:, :128], 1.0)
                        sv = sO[:].rearrange("p (w d) -> p w d", d=4)
                        cv = cO[:].rearrange("p (w d) -> p w d", d=2)
                        m1 = wrk.tile([128, Wc], F32, tag="m1")
                        m2 = wrk.tile([128, Wc], F32, tag="m2")
                        bt = wrk.tile([128, Wc], F32, tag="bt")
                        if "dve" not in skip:
                            nc.vector.tensor_tensor(out=m1[:], in0=cv[:, :, 0],
                                                    in1=sv[:, :, 1], op=OP.mult)
                            nc.vector.tensor_tensor(out=m2[:], in0=cv[:, :, 1],
                                                    in1=sv[:, :, 2], op=OP.mult)
                            nc.vector.tensor_tensor(out=m1[:], in0=m1[:], in1=m2[:],
                                                    op=OP.max)
                            if l > 0:
                                nc.vector.tensor_tensor(
                                    out=m2[:], in0=rankB[:, ci * Wc:(ci + 1) * Wc],
                                    in1=sv[:, :, 3], op=OP.is_lt)
                                nc.vector.tensor_tensor(out=m1[:], in0=m1[:],
                                                        in1=m2[:], op=OP.mult)
                            nc.vector.tensor_tensor(out=bt[:], in0=m1[:],
                                                    in1=sv[:, :, 0], op=OP.mult)
                        else:
                            nc.vector.memset(m1[:, :128], 1.0)
                            nc.vector.memset(bt[:, :128], 1.0)

                        for e in (by_chunk.get(ci, []) if "pe" not in skip
                                  else [x for x in by_chunk.get(ci, [])
                                        if first_e.get(0 if x["ps_lo"] < 512 else 1) == id(x)
                                        or last_e.get(0 if x["ps_lo"] < 512 else 1) == id(x)]):
                            pl = e["ps_lo"]
                            tix = 0 if pl < 512 else 1
                            pb = pl - tix * 512
                            wdt = e["rhs_hi"] - e["rhs_lo"]
                            is_last = last_e.get(tix) == id(e)
                            is_first = first_e.get(tix) == id(e)
                            nc.tensor.matmul(
                                out=psU[tix][:, pb:pb + wdt], lhsT=ident[:],
                                rhs=bt[:, e["rhs_lo"]:e["rhs_hi"]],
                                start=is_first, stop=is_last,
                                skip_group_check=True)
                            nc.tensor.matmul(
                                out=psS[tix][:, pb:pb + wdt], lhsT=ident[:],
                                rhs=m1[:, e["rhs_lo"]:e["rhs_hi"]],
                                start=is_first, stop=is_last,
                                skip_group_check=True)

                    # per-vertex tail
                    Svec = tiny.tile([128, nv_eq], F32, tag="svec")
                    for tix in range(len(psU)):
                        c0 = tix * 512
                        cw = psS[tix].shape[1]
                        nc.vector.tensor_scalar(out=Svec[:, c0:c0 + cw],
                                                in0=psS[tix][:], scalar1=1e-16,
                                                scalar2=None, op0=OP.add)
                    rS = tiny.tile([128, nv_eq], F32, tag="rsv")
                    nc.vector.reciprocal(out=rS[:], in_=Svec[:])
                    outv = tiny.tile([128, nv_eq], F32, tag="outv")
                    for tix in range(len(psU)):
                        c0 = tix * 512
                        cw = psU[tix].shape[1]
                        nc.vector.tensor_tensor(out=outv[:, c0:c0 + cw],
                                                in0=psU[tix][:],
                                                in1=rS[:, c0:c0 + cw], op=OP.mult)
                    h = tiny.tile([128, nv_eq], F32, tag="h")
                    nc.scalar.activation(out=h[:], in_=outv[:], func=AF.Relu,
                                         bias=bRep[l][:])
                    thrD = tiny.tile([128, nv_eq], F32, tag="thrd")
                    nc.gpsimd.ap_gather(
                        out_ap=thrD[:].rearrange("p (w d) -> p w d", d=1),
                        in_ap=kTab[l][:].rearrange("p (s d) -> p s d", d=1),
                        idxs_ap=aDstW[:], channels=128, num_elems=16, d=1,
                        num_idxs=nv_eq)
                    alive = tiny.tile([128, nv_eq], F32, tag="alive")
                    nc.vector.tensor_tensor(out=alive[:], in0=rankDstB[:],
                                            in1=thrD[:], op=OP.is_lt)
                    nc.vector.tensor_tensor(out=h[:], in0=h[:], in1=alive[:],
                                            op=OP.mult)
                    nc.vector.tensor_reduce(partials[:, l:l + 1], h[:], AX.X, OP.add)
                    nc.vector.tensor_reduce(partials[:, 3 + l:4 + l], h[:], AX.X, OP.max)

                # ---------------- combine + MLP ----------------
                if not tail:
                    continue
                cc_in = dram.tile([128, 8], F32)
                cc_out = dram.tile([NCORES * 128, 8], F32)
                nc.sync.dma_start(out=cc_in[:], in_=partials[:])
                nc.gpsimd.collective_compute(
                    "AllGather", OP.bypass,
                    replica_groups=[list(range(NCORES))],
                    ins=[cc_in[:].opt()], outs=[cc_out[:].opt()])
                allp = tiny.tile([128, NCORES * 8], F32, tag="allp")
                nc.sync.dma_start(
                    out=allp[:],
                    in_=bass.AP(cc_out[:].tensor, cc_out[:].offset,
                                [[8, 128], [1024, NCORES], [1, 8]]))
                comb = tiny.tile([128, 8], F32, tag="comb")
                nc.vector.memset(comb[:], 0.0)
                ab = allp[:]
                nc.vector.tensor_reduce(
                    comb[:, 0:3],
                    bass.AP(ab.tensor, ab.offset,
                            [list(ab.ap[0]), [1, 3], [8, NCORES]]),
                    AX.X, OP.add)
                nc.vector.tensor_reduce(
                    comb[:, 3:6],
                    bass.AP(ab.tensor, ab.offset + 3,
                            [list(ab.ap[0]), [1, 3], [8, NCORES]]),
                    AX.X, OP.max)
                shf = tiny.tile([128, 8], F32, tag="shf")
                for sh in (64, 32, 16):
                    nc.sync.dma_start(out=shf[:sh, :], in_=comb[sh:2 * sh, :])
                    nc.vector.tensor_tensor(out=comb[:sh, 0:3], in0=comb[:sh, 0:3],
                                            in1=shf[:sh, 0:3], op=OP.add)
                    nc.vector.tensor_tensor(out=comb[:sh, 3:6], in0=comb[:sh, 3:6],
                                            in1=shf[:sh, 3:6], op=OP.max)
                for l in range(3):
                    nc.vector.tensor_scalar(out=comb[:16, l:l + 1],
                                            in0=comb[:16, l:l + 1],
                                            scalar1=1.0 / QUOTAS[l], scalar2=None,
                                            op0=OP.mult)
                gcol = tiny.tile([96, 1], F32, tag="gcol")
                for l in range(3):
                    nc.sync.dma_start(out=gcol[32 * l:32 * l + 16, :],
                                      in_=comb[:16, l:l + 1])
                    nc.sync.dma_start(out=gcol[32 * l + 16:32 * l + 32, :],
                                      in_=comb[:16, 3 + l:4 + l])
                z1_ps = ps_small([1, 16])
                nc.tensor.matmul(out=z1_ps[:], lhsT=gcol[:], rhs=l1w[:],
                                 start=True, stop=True)
                h1 = tiny.tile([1, 16], F32, tag="h1")
                nc.vector.tensor_tensor(out=h1[:], in0=z1_ps[:], in1=l1b[:], op=OP.add)
                nc.scalar.activation(out=h1[:], in_=h1[:], func=AF.Relu)
                h1c_ps = ps_small([16, 1])
                nc.tensor.transpose(out=h1c_ps[:], in_=h1[:], identity=ident[:1, :1])
                h1c = tiny.tile([16, 1], F32, tag="h1c")
                nc.vector.tensor_copy(out=h1c[:], in_=h1c_ps[:])
                z2_ps = ps_small([1, 4])
                nc.tensor.matmul(out=z2_ps[:], lhsT=h1c[:], rhs=l2w[:],
                                 start=True, stop=True)
                h2 = tiny.tile([1, 4], F32, tag="h2")
                nc.vector.tensor_tensor(out=h2[:], in0=z2_ps[:], in1=l2b[:], op=OP.add)
                nc.scalar.activation(out=h2[:], in_=h2[:], func=AF.Relu)
                h2c_ps = ps_small([4, 1])
                nc.tensor.transpose(out=h2c_ps[:], in_=h2[:], identity=ident[:1, :1])
                h2c = tiny.tile([4, 1], F32, tag="h2c")
                nc.vector.tensor_copy(out=h2c[:], in_=h2c_ps[:])
                z3_ps = ps_small([1, 1])
                nc.tensor.matmul(out=z3_ps[:], lhsT=h2c[:], rhs=l3w[:],
                                 start=True, stop=True)
                z3 = tiny.tile([1, 1], F32, tag="z3")
                nc.vector.tensor_tensor(out=z3[:], in0=z3_ps[:], in1=l3b[:], op=OP.add)
                sig = tiny.tile([1, 1], F32, tag="sig")
                nc.scalar.activation(out=sig[:], in_=z3[:], func=AF.Sigmoid)
                nc.sync.dma_start(out=dout.ap(), in_=sig[:])
            if not tail:
                nc.sync.dma_start(out=dout.ap(), in_=partials[:1, :1])

    nc.finalize()
    return nc


# ------------------------------------------------------------------ driver

_CACHE = {}


def kernel(**inputs):
    in_maps_nc, meta = host_prep(inputs["node_ids"], inputs["edge_index"],
                                 inputs["edge_type"])
    shared = pack_model_inputs(inputs, meta["cnt"])
    in_maps = [{**m, **shared} for m in in_maps_nc]

    key = (meta["Lp"], meta["nv_eq"], tuple(meta["n_k"]))
    if key not in _CACHE:
        _CACHE[key] = build_program(meta)
    nc = _CACHE[key]

    res = run_bass_kernel_spmd(nc, in_maps, core_ids=list(range(NCORES)))
    return np.asarray(res.results[0]["out"], np.float32)



# revision 6
# speedup vs baseline: 488.3543x; 1.1725x over previous
"""Trainium2 Bass kernel for nn_DetectModel (RGAT x3 + TopK pool + MLP).

ap_gather costs ~28ns/index regardless of payload width, so per-edge work
is restructured to minimize gather *indices*:
  - One fused table index c3 = (a_dst*114 + t)*10 + a_src (11400 entries).
  - Phase A: one d=4 fp16 gather/edge serves layers 0+1 ((U0,P0,U1,P1) where
    U_l = Mt_l[s]*P_l, P_l = exp-lrelu attention factor).
  - Phase B: one d=2 fp16 gather/edge for layer 2.
  - Rank/alive masks computed in wrapped [128, Lp/16] space (no gather),
    expanded to [128, Lp] via a DRAM round-trip DMA transpose.
  - Tables built at full 128-partition width from replicated [*,1140] factors.
  - fp16 edge streams + fp16 identity PE matmuls into f32 PSUM.
  - Inputs shipped wrapped/int16+f32 (~0.7MB/core vs ~5.6MB in v1).
"""
import numpy as np

import concourse.bass as bass
import concourse.bacc as bacc
import concourse.mybir as mybir
import concourse.tile as tile
from concourse.bass_utils import run_bass_kernel_spmd
from concourse.masks import make_identity

F32 = mybir.dt.float32
F16 = mybir.dt.float16
I16 = mybir.dt.int16
AF = mybir.ActivationFunctionType
OP = mybir.AluOpType
AX = mybir.AxisListType

N0, N1, N2 = 50000, 40000, 32000
E = 600000
D = 16
R = 114
NA = 10
NCORES = 8
G = 8
NB = NCORES * G

NE3 = 11408            # c3 table slots; dead index = 11400
DEAD3 = 11400
QUOTAS = (N0, N1, N2)
TQ3 = ((0, 380), (380, 760), (760, 1140))


# ---------------------------------------------------------------- host prep

def host_prep(node_ids, edge_index, edge_type):
    a = np.asarray(node_ids).astype(np.int64)
    src = np.asarray(edge_index)[0].astype(np.int64)
    dst = np.asarray(edge_index)[1].astype(np.int64)
    t = np.asarray(edge_type).astype(np.int64)
    n0 = a.shape[0]

    cnt = np.bincount(a, minlength=NA).astype(np.int64)
    order_a = np.argsort(a, kind="stable")
    rank = np.empty(n0, np.int64)
    starts = np.zeros(NA + 1, np.int64)
    np.cumsum(cnt, out=starts[1:])
    rank[order_a] = np.arange(n0) - starts[a[order_a]]

    deg = np.bincount(dst, minlength=n0)
    e_order = np.argsort(dst, kind="stable")
    vstart = np.zeros(n0 + 1, np.int64)
    np.cumsum(deg, out=vstart[1:])

    vorder = np.argsort(-deg, kind="stable")
    nrows = (n0 + NB - 1) // NB
    pad_n = nrows * NB
    vpad = np.full(pad_n, -1, np.int64)
    vpad[:n0] = vorder
    grid = vpad.reshape(nrows, NB)
    grid[1::2] = grid[1::2, ::-1]
    buckets = [grid[:, b][grid[:, b] >= 0] for b in range(NB)]

    nv_eq = max(len(b) for b in buckets)
    nv_eq = ((nv_eq + 15) // 16) * 16

    maxdeg = int(deg.max()) if n0 else 0
    n_k = []
    for k in range(maxdeg):
        w = max(int((deg[b] > k).sum()) for b in buckets)
        if w == 0:
            break
        n_k.append(w)
    n_k[0] = nv_eq
    offs = np.zeros(len(n_k) + 1, np.int64)
    np.cumsum(n_k, out=offs[1:])
    L = int(offs[-1])
    Lp = ((L + 255) // 256) * 256

    c3_e = ((a[dst] * R + t) * NA + a[src]).astype(np.int64)
    z_src_e = (a[src] * 8192 + rank[src]).astype(np.float32)

    def wrap(x, width):
        return (x.reshape(G, width // 16, 16)
                 .transpose(0, 2, 1)
                 .reshape(G * 16, width // 16))

    in_maps = []
    for n in range(NCORES):
        c3Idx = np.full((G, Lp), DEAD3, np.int64)
        zS = np.full((G, Lp), 8191.0, np.float32)
        zD = np.full((G, nv_eq), 8191.0, np.float32)
        for g in range(G):
            bl = buckets[n * G + g]
            nb = len(bl)
            zD[g, :nb] = (a[bl] * 8192 + rank[bl]).astype(np.float32)
            dg = deg[bl]
            for k in range(len(n_k)):
                rsel = np.nonzero(dg > k)[0]
                if len(rsel) == 0:
                    continue
                eids = e_order[vstart[bl[rsel]] + k]
                cols = offs[k] + rsel
                c3Idx[g, cols] = c3_e[eids]
                zS[g, cols] = z_src_e[eids]

        m = {
            "c3IdxW": np.ascontiguousarray(wrap(c3Idx, Lp).astype(np.int16)),
            "zSrcB": np.ascontiguousarray(zS.reshape(128, Lp // 16)),
            "zDstB": np.ascontiguousarray(zD.reshape(128, nv_eq // 16)),
        }
        in_maps.append(m)

    meta = {"n_k": n_k, "offs": offs, "L": L, "Lp": Lp, "nv_eq": nv_eq,
            "cnt": cnt}
    return in_maps, meta


def pack_model_inputs(inp, cnt):
    f = lambda x: np.ascontiguousarray(np.asarray(x, np.float32))
    emb = f(inp["emb"])
    sh = {
        "embT": np.ascontiguousarray(emb.T),
        "pw0": f(inp["pw0"]).reshape(16, 1),
        "pw1": f(inp["pw1"]).reshape(16, 1),
        "cntRep": np.ascontiguousarray(np.tile(cnt.astype(np.float32), (NA, 1))),
        "cntCol": cnt.astype(np.float32).reshape(NA, 1),
        "l1w": f(inp["l1w"]), "l1b": f(inp["l1b"]).reshape(1, 16),
        "l2w": f(inp["l2w"]), "l2b": f(inp["l2b"]).reshape(1, 4),
        "l3w": f(inp["l3w"]), "l3b": f(inp["l3b"]).reshape(1, 1),
    }
    for l in range(3):
        W = f(inp[f"W{l}"])
        sh[f"Wst{l}"] = np.ascontiguousarray(
            W.transpose(1, 0, 2).reshape(16, R * 16))
        sh[f"qv{l}"] = f(inp[f"q{l}"]).reshape(16, 1)
        sh[f"kv{l}"] = f(inp[f"k{l}"]).reshape(16, 1)
        sh[f"bv{l}"] = f(inp[f"b{l}"]).reshape(16, 1)
    return sh


def seg_matmul_list(meta, n_chunks):
    n_k, offs, Lp = meta["n_k"], meta["offs"], meta["Lp"]
    Wc = Lp // n_chunks
    out = []
    for k, nk in enumerate(n_k):
        lo, hi = int(offs[k]), int(offs[k] + nk)
        p = lo
        pieces = []
        while p < hi:
            q = min(hi, (p // Wc + 1) * Wc)
            r0, r1 = p - lo, q - lo
            if r0 < 512 < r1:
                pieces += [(p, lo + 512), (lo + 512, q)]
            else:
                pieces.append((p, q))
            p = q
        for (p0, p1) in pieces:
            out.append({
                "chunk": p0 // Wc,
                "rhs_lo": p0 - (p0 // Wc) * Wc,
                "rhs_hi": p1 - (p0 // Wc) * Wc,
                "ps_lo": p0 - lo,
                "start": (k == 0),
            })
    return out


# ------------------------------------------------------------- bass builder

def build_program(meta, n_chunks=16, repeats=1, skip=(), num_devices=NCORES,
                  tail=True):
    n_k, Lp, nv_eq = meta["n_k"], meta["Lp"], meta["nv_eq"]
    Wc = Lp // n_chunks
    assert Wc % 16 == 0
    W16 = Lp // 16
    NV16 = nv_eq // 16
    segs = seg_matmul_list(meta, n_chunks)
    by_chunk = {}
    for e in segs:
        by_chunk.setdefault(e["chunk"], []).append(e)

    nc = bacc.Bacc("TRN2", target_bir_lowering=False, debug=False,
                   num_devices=num_devices)

    din = {}
    din["c3IdxW"] = nc.dram_tensor("c3IdxW", [128, W16], I16, kind="ExternalInput")
    din["zSrcB"] = nc.dram_tensor("zSrcB", [128, W16], F32, kind="ExternalInput")
    din["zDstB"] = nc.dram_tensor("zDstB", [128, NV16], F32, kind="ExternalInput")
    for name, shape in [("embT", [16, NA]), ("pw0", [16, 1]), ("pw1", [16, 1]),
                        ("cntRep", [NA, NA]), ("cntCol", [NA, 1]),
                        ("l1w", [96, 16]), ("l1b", [1, 16]),
                        ("l2w", [16, 4]), ("l2b", [1, 4]),
                        ("l3w", [4, 1]), ("l3b", [1, 1])]:
        din[name] = nc.dram_tensor(name, shape, F32, kind="ExternalInput")
    for l in range(3):
        din[f"Wst{l}"] = nc.dram_tensor(f"Wst{l}", [16, R * 16], F32, kind="ExternalInput")
        for nm in ("qv", "kv", "bv"):
            din[f"{nm}{l}"] = nc.dram_tensor(f"{nm}{l}", [16, 1], F32, kind="ExternalInput")
    dout = nc.dram_tensor("out", [1], F32, kind="ExternalOutput")

    with tile.TileContext(nc) as tc:
        with (
            tc.tile_pool(name="stat", bufs=1) as stat,
            tc.tile_pool(name="tabp", bufs=1) as tabp,
            tc.tile_pool(name="wrk", bufs=2) as wrk,
            tc.tile_pool(name="tiny", bufs=1) as tiny,
            tc.tile_pool(name="dram", bufs=1, space="DRAM") as dram,
        ):
            # ---------- static loads ----------
            c3Idx = stat.tile([128, W16], I16)
            zSrcB = stat.tile([128, W16], F32)
            zDstB = stat.tile([128, NV16], F32)
            embT = stat.tile([16, NA], F32)
            cntRep = stat.tile([NA, NA], F32)
            cntCol = stat.tile([NA, 1], F32)
            l1w = stat.tile([96, 16], F32)
            l1b = stat.tile([1, 16], F32)
            l2w = stat.tile([16, 4], F32)
            l2b = stat.tile([1, 4], F32)
            l3w = stat.tile([4, 1], F32)
            l3b = stat.tile([1, 1], F32)
            pw = [stat.tile([16, 1], F32, tag=f"pw{i}", name=f"pw{i}") for i in range(2)]
            qv = [stat.tile([16, 1], F32, tag=f"qv{i}", name=f"qvt{i}") for i in range(3)]
            kv = [stat.tile([16, 1], F32, tag=f"kv{i}", name=f"kvt{i}") for i in range(3)]
            for tl, name in ([(c3Idx, "c3IdxW"), (zSrcB, "zSrcB"),
                              (zDstB, "zDstB"),
                              (embT, "embT"), (cntRep, "cntRep"), (cntCol, "cntCol"),
                              (l1w, "l1w"), (l1b, "l1b"), (l2w, "l2w"), (l2b, "l2b"),
                              (l3w, "l3w"), (l3b, "l3b"),
                              (pw[0], "pw0"), (pw[1], "pw1")]
                             + [(qv[l], f"qv{l}") for l in range(3)]
                             + [(kv[l], f"kv{l}") for l in range(3)]):
                nc.sync.dma_start(out=tl[:], in_=din[name].ap())
            bRep = [stat.tile([128, 1], F32, tag=f"bR{i}", name=f"bR{i}") for i in range(3)]
            for l in range(3):
                nc.sync.dma_start(out=bRep[l][:],
                                  in_=bass.AP(din[f"bv{l}"], 0, [[0, 8], [1, 16]]))

            ident = stat.tile([128, 128], F32)
            make_identity(nc, ident[:])
            identF = stat.tile([128, 128], F16)
            make_identity(nc, identF[:])
            ones1 = stat.tile([1, 128], F32)
            nc.vector.memset(ones1[:], 1.0)

            def colbc(col_ap, n, m):
                return bass.AP(col_ap.tensor, col_ap.offset,
                               [list(col_ap.ap[0]), [0, m]])

            # ---------- prolog: s0, s1, keep counts, kTab ----------
            with tc.tile_pool(name="psP", bufs=1, space="PSUM") as psP:
                def ps_small(shape, tg="tps"):
                    return psP.tile(shape, F32, space="PSUM", tag=tg, name="pstiny")

                def rnorm_of(pwt):
                    nrm = ps_small([1, 1])
                    nc.tensor.matmul(out=nrm[:], lhsT=pwt[:], rhs=pwt[:],
                                     start=True, stop=True)
                    sq = tiny.tile([1, 1], F32, tag="sq")
                    nc.scalar.activation(out=sq[:], in_=nrm[:], func=AF.Sqrt)
                    rn = tiny.tile([1, 1], F32, tag="rn")
                    nc.vector.reciprocal(out=rn[:], in_=sq[:])
                    rrep = ps_small([NA, 1])
                    nc.tensor.matmul(out=rrep[:], lhsT=ones1[:, :NA], rhs=rn[:],
                                     start=True, stop=True)
                    rs = tiny.tile([NA, 1], F32, tag="rs10")
                    nc.vector.tensor_copy(out=rs[:], in_=rrep[:])
                    return rs

                rn0 = rnorm_of(pw[0])
                s0 = stat.tile([NA, 1], F32)
                dot0 = ps_small([NA, 1])
                nc.tensor.matmul(out=dot0[:], lhsT=embT[:], rhs=pw[0][:],
                                 start=True, stop=True)
                nc.scalar.activation(out=s0[:], in_=dot0[:], func=AF.Tanh, scale=rn0[:])
                rn1 = rnorm_of(pw[1])
                dot1 = ps_small([NA, 1])
                nc.tensor.matmul(out=dot1[:], lhsT=embT[:], rhs=pw[1][:],
                                 start=True, stop=True)
                d1s = tiny.tile([NA, 1], F32, tag="d1s")
                nc.vector.tensor_tensor(out=d1s[:], in0=dot1[:], in1=s0[:], op=OP.mult)
                s1 = stat.tile([NA, 1], F32)
                nc.scalar.activation(out=s1[:], in_=d1s[:], func=AF.Tanh, scale=rn1[:])

                cc = [stat.tile([NA, 1], F32, tag=f"cc{i}", name=f"cct{i}") for i in range(3)]
                nc.vector.memset(cc[0][:], 1.0)
                nc.vector.tensor_copy(out=cc[1][:], in_=s0[:])
                nc.vector.tensor_tensor(out=cc[2][:], in0=s0[:], in1=s1[:], op=OP.mult)

                def keep_counts(score_col, quota, prev_col, kk):
                    srow_ps = ps_small([1, NA])
                    nc.tensor.transpose(out=srow_ps[:], in_=score_col[:],
                                        identity=ident[:NA, :NA])
                    srow = tiny.tile([1, NA], F32, tag="srow")
                    nc.vector.tensor_copy(out=srow[:], in_=srow_ps[:])
                    srep_ps = ps_small([NA, NA])
                    nc.tensor.matmul(out=srep_ps[:], lhsT=ones1[:, :NA], rhs=srow[:],
                                     start=True, stop=True)
                    gt = tiny.tile([NA, NA], F32, tag="gt")
                    nc.vector.tensor_tensor(out=gt[:], in0=srep_ps[:],
                                            in1=colbc(score_col[:], NA, NA), op=OP.is_gt)
                    nc.vector.tensor_tensor(out=gt[:], in0=gt[:], in1=cntRep[:], op=OP.mult)
                    cum = tiny.tile([NA, 1], F32, tag="cum")
                    nc.vector.tensor_reduce(cum[:], gt[:], AX.X, OP.add)
                    nc.vector.tensor_scalar(out=kk[:], in0=cum[:], scalar1=-1.0,
                                            scalar2=float(quota), op0=OP.mult, op1=OP.add)
                    nc.vector.tensor_scalar(out=kk[:], in0=kk[:], scalar1=0.0,
                                            scalar2=None, op0=OP.max)
                    nc.vector.tensor_tensor(out=kk[:], in0=kk[:], in1=prev_col[:], op=OP.min)

                kcol = [cntCol,
                        stat.tile([NA, 1], F32, tag="k1", name="k1"),
                        stat.tile([NA, 1], F32, tag="k2", name="k2")]
                keep_counts(s0, N1, kcol[0], kcol[1])
                keep_counts(s1, N2, kcol[1], kcol[2])

                kTab = []
                for l in range(3):
                    kr = stat.tile([1, 16], F32, tag=f"kr{l}", name=f"kr{l}")
                    nc.vector.memset(kr[:], 0.0)
                    kr_ps = ps_small([1, NA])
                    nc.tensor.transpose(out=kr_ps[:], in_=kcol[l][:],
                                        identity=ident[:NA, :NA])
                    nc.vector.tensor_copy(out=kr[:, :NA], in_=kr_ps[:])
                    kt_ps = ps_small([128, 16], tg="ktps")
                    nc.tensor.matmul(out=kt_ps[:], lhsT=ones1[:], rhs=kr[:],
                                     start=True, stop=True)
                    kt = stat.tile([128, 16], F32, tag=f"kt{l}", name=f"ktt{l}")
                    nc.vector.tensor_copy(out=kt[:], in_=kt_ps[:])
                    kTab.append(kt)

            # ---------- alive via interval counting on z = a*8192 + rank ----------
            # alive(z; k) = sum_a [z < a*8192 + k[a]] - sum_a [z < a*8192]
            baseTab = stat.tile([128, 16], F32)
            for a in range(NA):
                nc.vector.memset(baseTab[:, a:a + 1], float(a * 8192))
            KTab = []
            for l in range(3):
                ktl = stat.tile([128, 16], F32, tag=f"KT{l}", name=f"KT{l}")
                nc.vector.tensor_tensor(out=ktl[:, :NA], in0=kTab[l][:, :NA],
                                        in1=baseTab[:, :NA], op=OP.add)
                KTab.append(ktl)

            def cbc(t, col, m):
                return bass.AP(t[:].tensor, t[:].offset + col,
                               [list(t[:].ap[0]), [0, m]])

            def count_lt(zt, ktab, ncols, dtype, tag):
                acc = tiny.tile([128, ncols], dtype, tag=f"clt_{tag}",
                                name=f"clt{tag}")
                stp = tiny.tile([128, ncols], dtype, tag=f"stp_{tag}",
                                name=f"stp{tag}")
                for a in range(NA):
                    dst = acc if a == 0 else stp
                    nc.vector.tensor_tensor(out=dst[:], in0=zt[:],
                                            in1=cbc(ktab, a, ncols), op=OP.is_lt)
                    if a > 0:
                        nc.vector.tensor_tensor(out=acc[:], in0=acc[:],
                                                in1=stp[:], op=OP.add)
                return acc

            c3mD = stat.tile([128, W16], F32)
            nc.vector.tensor_copy(out=c3mD[:], in_=c3Idx[:])
            nc.vector.tensor_scalar(out=c3mD[:], in0=c3mD[:],
                                    scalar1=float(-DEAD3), scalar2=None,
                                    op0=OP.add)
            baseCntS = stat.tile([128, W16], F16)
            tmpS = count_lt(zSrcB, baseTab, W16, F16, "s")
            nc.vector.tensor_copy(out=baseCntS[:], in_=tmpS[:])
            baseCntD = stat.tile([128, NV16], F32)
            tmpD = count_lt(zDstB, baseTab, NV16, F32, "d")
            nc.vector.tensor_copy(out=baseCntD[:], in_=tmpD[:])

            # dram scratch for wrapped->expanded transposes
            scrW = dram.tile([128, W16], F16)
            scrD = dram.tile([128, NV16], F32)

            def expand(wr_tile, ncols, out_tile, dscr):
                # SBUF block [128, ncols] -> DRAM -> SBUF expanded [128, 16*ncols]
                # (partition 16g+p holds cols [p*ncols, (p+1)*ncols) of bucket g)
                nc.sync.dma_start(out=dscr[:, :ncols], in_=wr_tile[:])
                for g in range(G):
                    src = bass.AP(dscr[:].tensor,
                                  dscr[:].offset + g * 16 * dscr.shape[1],
                                  [[0, 16], [dscr.shape[1], 16], [1, ncols]])
                    o = out_tile[16 * g:16 * (g + 1), :]
                    dst = bass.AP(o.tensor, o.offset,
                                  [list(o.ap[0]), [ncols, 16], [1, ncols]])
                    nc.sync.dma_start(out=dst, in_=src)

            def build_alive(zt, l, baseCnt, ncols, out_dtype, tag):
                aliveW = count_lt(zt, KTab[l], ncols, out_dtype, tag)
                nc.vector.tensor_tensor(out=aliveW[:], in0=aliveW[:],
                                        in1=baseCnt[:], op=OP.subtract)
                return aliveW

            # static dst mask for layer 0 (rank < cnt[a]: real-vertex indicator)
            aliveD0 = stat.tile([128, nv_eq], F32)
            alD0w = build_alive(zDstB, 0, baseCntD, NV16, F32, "d")
            expand(alD0w, NV16, aliveD0, scrD)

            partials = stat.tile([128, 8], F32)
            nc.vector.memset(partials[:], 0.0)

            # big fused table + per-layer factor tiles
            tabT = tabp.tile([128, NE3 * 4], F16)
            MtS = tabp.tile([128, 1140], F16, tag="mts")
            EQ = tabp.tile([128, 1140], F16, tag="eq")
            E2Q = tabp.tile([128, 1140], F16, tag="e2q")
            EK = tabp.tile([128, 1140], F16, tag="ek")
            E2K = tabp.tile([128, 1140], F16, tag="e2k")
            wst = tabp.tile([16, R * 16], F32, tag="wst")
            aliveB = tabp.tile([128, Lp], F16, tag="aliveB")
            aliveD = [tabp.tile([128, nv_eq], F32, tag=f"alivD{l}",
                                name=f"alivD{l}") for l in (1, 2)]

            def replicate16(t, width):
                # [0:16, :width] -> all 128 partitions (doubling DMAs)
                for p in (16, 32, 64):
                    nc.sync.dma_start(out=t[p:2 * p, :width], in_=t[0:p, :width])

            def build_layer_factors(l, psT):
                # emblT = embT * cc[l] (broadcast over features)
                crow_ps = psT.tile([1, NA], F32, space="PSUM", tag="crow")
                nc.tensor.transpose(out=crow_ps[:], in_=cc[l][:],
                                    identity=ident[:NA, :NA])
                crow = tiny.tile([1, NA], F32, tag="crow")
                nc.vector.tensor_copy(out=crow[:], in_=crow_ps[:])
                crep_ps = psT.tile([16, NA], F32, space="PSUM", tag="crep")
                nc.tensor.matmul(out=crep_ps[:], lhsT=ones1[:, :16],
                                 rhs=crow[:], start=True, stop=True)
                emblT = tiny.tile([16, NA], F32, tag="emblT")
                nc.vector.tensor_tensor(out=emblT[:], in0=embT[:],
                                        in1=crep_ps[:], op=OP.mult)

                nc.sync.dma_start(out=wst[:], in_=din[f"Wst{l}"].ap())
                qvR = tiny.tile([16, 16], F16, tag="qvR")
                kvR = tiny.tile([16, 16], F16, tag="kvR")
                nc.vector.tensor_copy(out=qvR[:], in_=colbc(qv[l][:], 16, 16))
                nc.vector.tensor_copy(out=kvR[:], in_=colbc(kv[l][:], 16, 16))

                # Mt [16, 1140] (s = t*10 + a_src)
                for blk in range(3):
                    mt_ps = psT.tile([16, 512], F32, space="PSUM", tag="mt")
                    t0 = blk * 38
                    t1 = min(R, t0 + 38)
                    for ti in range(t0, t1):
                        col = (ti - t0) * NA
                        nc.tensor.matmul(
                            out=mt_ps[:, col:col + NA],
                            lhsT=wst[:, ti * 16:(ti + 1) * 16],
                            rhs=emblT[:],
                            start=True, stop=True, skip_group_check=True)
                    cw = (t1 - t0) * NA
                    nc.vector.tensor_copy(out=MtS[:16, t0 * NA:t0 * NA + cw],
                                          in_=mt_ps[:, :cw])

                # Tq/Tk + exps (s-space)
                for (c0, c1) in TQ3:
                    tkp = psT.tile([16, 512], F32, space="PSUM", tag="tqk")
                    nc.tensor.matmul(out=tkp[:, :c1 - c0], lhsT=kvR[:],
                                     rhs=MtS[:16, c0:c1], start=True, stop=True,
                                     skip_group_check=True)
                    nc.scalar.activation(out=EK[:16, c0:c1], in_=tkp[:, :c1 - c0],
                                         func=AF.Exp)
                    nc.scalar.activation(out=E2K[:16, c0:c1], in_=tkp[:, :c1 - c0],
                                         func=AF.Exp, scale=0.2)
                    tqp = psT.tile([16, 512], F32, space="PSUM", tag="tqk2")
                    nc.tensor.matmul(out=tqp[:, :c1 - c0], lhsT=qvR[:],
                                     rhs=MtS[:16, c0:c1], start=True, stop=True,
                                     skip_group_check=True)
                    nc.scalar.activation(out=EQ[:16, c0:c1], in_=tqp[:, :c1 - c0],
                                         func=AF.Exp)
                    nc.scalar.activation(out=E2Q[:16, c0:c1], in_=tqp[:, :c1 - c0],
                                         func=AF.Exp, scale=0.2)
                for tt in (MtS, EQ, E2Q, EK, E2K):
                    replicate16(tt, 1140)

            def write_planes(d_total, slot_u, slot_p):
                # P = max(EQ[c2]*EK[s], E2Q[c2]*E2K[s]); U = Mt[s]*P, written
                # interleaved into tabT at stride d_total (full 128-partition).
                Ablk = tiny.tile([128, 2280], F16, tag="Ablk")
                Bblk = tiny.tile([128, 2280], F16, tag="Bblk")
                for ab in range(5):
                    a0 = 2 * ab
                    def qview(t):
                        return bass.AP(t[:].tensor, t[:].offset + a0,
                                       [list(t[:].ap[0]), [1, 2], [NA, R], [0, NA]])
                    def sview(t):
                        return bass.AP(t[:].tensor, t[:].offset,
                                       [list(t[:].ap[0]), [0, 2], [NA, R], [1, NA]])
                    nc.vector.tensor_tensor(out=Ablk[:], in0=qview(EQ),
                                            in1=sview(EK), op=OP.mult)
                    nc.vector.tensor_tensor(out=Bblk[:], in0=qview(E2Q),
                                            in1=sview(E2K), op=OP.mult)
                    nc.vector.tensor_tensor(out=Ablk[:], in0=Ablk[:], in1=Bblk[:],
                                            op=OP.max)
                    base = a0 * 1140 * d_total
                    pout = bass.AP(tabT[:].tensor, tabT[:].offset + base + slot_p,
                                   [list(tabT[:].ap[0]), [d_total, 2280]])
                    nc.vector.tensor_copy(out=pout, in_=Ablk[:])
                    uout = bass.AP(tabT[:].tensor, tabT[:].offset + base + slot_u,
                                   [list(tabT[:].ap[0]), [d_total, 2280]])
                    nc.vector.tensor_tensor(out=uout, in0=sview(MtS), in1=Ablk[:],
                                            op=OP.mult)

            def run_phase(psX, d_total, layers, tail_specs, idx_tile=None):
                # layers: list of (slot_u, slot_p, aliveB or None) streams
                w1 = min(512, nv_eq)
                w2 = nv_eq - w1
                acc = {}
                for li, _ in enumerate(layers):
                    for nm in ("U", "S"):
                        tiles = [psX.tile([128, w1], F32, space="PSUM",
                                          tag=f"ps{nm}{li}0", name=f"ps{nm}{li}0")]
                        if w2 > 0:
                            tiles.append(psX.tile([128, w2], F32, space="PSUM",
                                                  tag=f"ps{nm}{li}1", name=f"ps{nm}{li}1"))
                        acc[(li, nm)] = tiles

                last_e = {}
                first_e = {}
                for ci in range(n_chunks):
                    for e in by_chunk.get(ci, []):
                        tx = 0 if e["ps_lo"] < 512 else 1
                        last_e[tx] = id(e)
                        if tx not in first_e:
                            first_e[tx] = id(e)

                for ci in range(n_chunks):
                    i0 = ci * (Wc // 16)
                    sO = wrk.tile([128, Wc * d_total], F16, tag=f"sO{d_total}",
                                  name="sO")
                    if "gather" not in skip:
                        nc.gpsimd.ap_gather(
                            out_ap=sO[:].rearrange("p (w d) -> p w d", d=d_total),
                            in_ap=tabT[:, :NE3 * d_total].rearrange(
                                "p (s d) -> p s d", d=d_total),
                            idxs_ap=(idx_tile if idx_tile is not None
                                     else c3Idx)[:, i0:i0 + Wc // 16],
                            channels=128, num_elems=NE3, d=d_total, num_idxs=Wc)
                    else:
                        nc.vector.memset(sO[:, :128], 1.0)
                    sv = sO[:].rearrange("p (w d) -> p w d", d=d_total)
                    streams = []
                    for li, (slot_u, slot_p, alB) in enumerate(layers):
                        if alB is None:
                            streams.append((sv[:, :, slot_u], sv[:, :, slot_p]))
                        else:
                            bt = wrk.tile([128, Wc], F16, tag=f"bt{li}",
                                          name=f"bt{li}")
                            m1 = wrk.tile([128, Wc], F16, tag=f"m1{li}",
                                          name=f"m1{li}")
                            als = alB[:, ci * Wc:(ci + 1) * Wc]
                            nc.vector.tensor_tensor(out=bt[:], in0=sv[:, :, slot_u],
                                                    in1=als, op=OP.mult)
                            nc.vector.tensor_tensor(out=m1[:], in0=sv[:, :, slot_p],
                                                    in1=als, op=OP.mult)
                            streams.append((bt[:], m1[:]))

                    for e in by_chunk.get(ci, []):
                        pl = e["ps_lo"]
                        tix = 0 if pl < 512 else 1
                        pb = pl - tix * 512
                        wdt = e["rhs_hi"] - e["rhs_lo"]
                        is_last = last_e.get(tix) == id(e)
                        is_first = first_e.get(tix) == id(e)
                        if "pe" in skip and not (is_first or is_last):
                            continue
                        for li, (bt, m1) in enumerate(streams):
                            nc.tensor.matmul(
                                out=acc[(li, "U")][tix][:, pb:pb + wdt],
                                lhsT=identF[:],
                                rhs=bt[:, e["rhs_lo"]:e["rhs_hi"]],
                                start=is_first, stop=is_last,
                                skip_group_check=True)
                            nc.tensor.matmul(
                                out=acc[(li, "S")][tix][:, pb:pb + wdt],
                                lhsT=identF[:],
                                rhs=m1[:, e["rhs_lo"]:e["rhs_hi"]],
                                start=is_first, stop=is_last,
                                skip_group_check=True)

                # tails
                for li, (l, alD) in enumerate(tail_specs):
                    psU = acc[(li, "U")]
                    psS = acc[(li, "S")]
                    Svec = tiny.tile([128, nv_eq], F32, tag="svec")
                    for tix in range(len(psU)):
                        c0 = tix * 512
                        cw = psS[tix].shape[1]
                        nc.vector.tensor_scalar(out=Svec[:, c0:c0 + cw],
                                                in0=psS[tix][:], scalar1=1e-16,
                                                scalar2=None, op0=OP.add)
                    nc.vector.reciprocal(out=Svec[:], in_=Svec[:])
                    h = tiny.tile([128, nv_eq], F32, tag="h")
                    for tix in range(len(psU)):
                        c0 = tix * 512
                        cw = psU[tix].shape[1]
                        nc.vector.tensor_tensor(out=h[:, c0:c0 + cw],
                                                in0=psU[tix][:],
                                                in1=Svec[:, c0:c0 + cw], op=OP.mult)
                    nc.scalar.activation(out=h[:], in_=h[:], func=AF.Relu,
                                         bias=bRep[l][:])
                    nc.vector.tensor_tensor(out=h[:], in0=h[:], in1=alD[:],
                                            op=OP.mult)
                    nc.vector.tensor_reduce(partials[:, l:l + 1], h[:], AX.X, OP.add)
                    nc.vector.tensor_reduce(partials[:, 3 + l:4 + l], h[:], AX.X,
                                            OP.max)

            # ---------------- repeats ----------------
            for _rep in range(repeats):
                # per-layer alive masks (dst for l=1,2; src for l=1,2)
                alW1 = build_alive(zSrcB, 1, baseCntS, W16, F16, "s")
                expand(alW1, W16, aliveB, scrW)
                alD1w = build_alive(zDstB, 1, baseCntD, NV16, F32, "d")
                expand(alD1w, NV16, aliveD[0], scrD)
                alD2w = build_alive(zDstB, 2, baseCntD, NV16, F32, "d")
                expand(alD2w, NV16, aliveD[1], scrD)

                # phase A: layers 0+1
                with tc.tile_pool(name=f"psT{_rep}", bufs=1, space="PSUM") as psT:
                    build_layer_factors(0, psT)
                    write_planes(4, 0, 1)
                    build_layer_factors(1, psT)
                    write_planes(4, 2, 3)
                    nc.vector.memset(tabT[:, DEAD3 * 4:], 0.0)
                with tc.tile_pool(name=f"psA{_rep}", bufs=1, space="PSUM") as psA:
                    run_phase(psA, 4,
                              [(0, 1, None), (2, 3, aliveB)],
                              [(0, aliveD0), (1, aliveD[0])])

                # phase B: layer 2 (d=2 table in the same tile)
                with tc.tile_pool(name=f"psU{_rep}", bufs=1, space="PSUM") as psT2:
                    build_layer_factors(2, psT2)
                alW2 = build_alive(zSrcB, 2, baseCntS, W16, F32, "s2")
                c3X = tiny.tile([128, W16], I16, tag="c3X")
                c3Xf = tiny.tile([128, W16], F32, tag="c3Xf")
                nc.vector.tensor_tensor(out=c3Xf[:], in0=c3mD[:], in1=alW2[:],
                                        op=OP.mult)
                nc.vector.tensor_scalar(out=c3X[:], in0=c3Xf[:],
                                        scalar1=float(DEAD3), scalar2=None,
                                        op0=OP.add)
                write_planes(2, 0, 1)
                nc.vector.memset(tabT[:, DEAD3 * 2:NE3 * 2], 0.0)
                with tc.tile_pool(name=f"psB{_rep}", bufs=1, space="PSUM") as psB:
                    run_phase(psB, 2,
                              [(0, 1, None)],
                              [(2, aliveD[1])], idx_tile=c3X)

                # ---------------- combine + MLP ----------------
                if not tail:
                    continue
                with tc.tile_pool(name=f"psM{_rep}", bufs=1, space="PSUM") as psM:
                    def ps_small2(shape, tg="tps"):
                        return psM.tile(shape, F32, space="PSUM", tag=tg, name="pstiny")

                    cc_in = dram.tile([128, 8], F32)
                    cc_out = dram.tile([NCORES * 128, 8], F32)
                    nc.sync.dma_start(out=cc_in[:], in_=partials[:])
                    nc.gpsimd.collective_compute(
                        "AllGather", OP.bypass,
                        replica_groups=[list(range(NCORES))],
                        ins=[cc_in[:].opt()], outs=[cc_out[:].opt()])
                    allp = tiny.tile([128, NCORES * 8], F32, tag="allp")
                    nc.sync.dma_start(
                        out=allp[:],
                        in_=bass.AP(cc_out[:].tensor, cc_out[:].offset,
                                    [[8, 128], [1024, NCORES], [1, 8]]))
                    comb = tiny.tile([128, 8], F32, tag="comb")
                    nc.vector.memset(comb[:], 0.0)
                    ab = allp[:]
                    nc.vector.tensor_reduce(
                        comb[:, 0:3],
                        bass.AP(ab.tensor, ab.offset,
                                [list(ab.ap[0]), [1, 3], [8, NCORES]]),
                        AX.X, OP.add)
                    nc.vector.tensor_reduce(
                        comb[:, 3:6],
                        bass.AP(ab.tensor, ab.offset + 3,
                                [list(ab.ap[0]), [1, 3], [8, NCORES]]),
                        AX.X, OP.max)
                    shf = tiny.tile([128, 8], F32, tag="shf")
                    for sh in (64, 32, 16):
                        nc.sync.dma_start(out=shf[:sh, :], in_=comb[sh:2 * sh, :])
                        nc.vector.tensor_tensor(out=comb[:sh, 0:3], in0=comb[:sh, 0:3],
                                                in1=shf[:sh, 0:3], op=OP.add)
                        nc.vector.tensor_tensor(out=comb[:sh, 3:6], in0=comb[:sh, 3:6],
                                                in1=shf[:sh, 3:6], op=OP.max)
                    for l in range(3):
                        nc.vector.tensor_scalar(out=comb[:16, l:l + 1],
                                                in0=comb[:16, l:l + 1],
                                                scalar1=1.0 / QUOTAS[l], scalar2=None,
                                                op0=OP.mult)
                    gcol = tiny.tile([96, 1], F32, tag="gcol")
                    for l in range(3):
                        nc.sync.dma_start(out=gcol[32 * l:32 * l + 16, :],
                                          in_=comb[:16, l:l + 1])
                        nc.sync.dma_start(out=gcol[32 * l + 16:32 * l + 32, :],
                                          in_=comb[:16, 3 + l:4 + l])
                    z1_ps = ps_small2([1, 16])
                    nc.tensor.matmul(out=z1_ps[:], lhsT=gcol[:], rhs=l1w[:],
                                     start=True, stop=True)
                    h1 = tiny.tile([1, 16], F32, tag="h1")
                    nc.vector.tensor_tensor(out=h1[:], in0=z1_ps[:], in1=l1b[:], op=OP.add)
                    nc.scalar.activation(out=h1[:], in_=h1[:], func=AF.Relu)
                    h1c_ps = ps_small2([16, 1], tg="h1c")
                    nc.tensor.transpose(out=h1c_ps[:], in_=h1[:], identity=ident[:1, :1])
                    h1c = tiny.tile([16, 1], F32, tag="h1c")
                    nc.vector.tensor_copy(out=h1c[:], in_=h1c_ps[:])
                    z2_ps = ps_small2([1, 4], tg="z2")
                    nc.tensor.matmul(out=z2_ps[:], lhsT=h1c[:], rhs=l2w[:],
                                     start=True, stop=True)
                    h2 = tiny.tile([1, 4], F32, tag="h2")
                    nc.vector.tensor_tensor(out=h2[:], in0=z2_ps[:], in1=l2b[:], op=OP.add)
                    nc.scalar.activation(out=h2[:], in_=h2[:], func=AF.Relu)
                    h2c_ps = ps_small2([4, 1], tg="h2c")
                    nc.tensor.transpose(out=h2c_ps[:], in_=h2[:], identity=ident[:1, :1])
                    h2c = tiny.tile([4, 1], F32, tag="h2c")
                    nc.vector.tensor_copy(out=h2c[:], in_=h2c_ps[:])
                    z3_ps = ps_small2([1, 1], tg="z3")
                    nc.tensor.matmul(out=z3_ps[:], lhsT=h2c[:], rhs=l3w[:],
                                     start=True, stop=True)
                    z3 = tiny.tile([1, 1], F32, tag="z3")
                    nc.vector.tensor_tensor(out=z3[:], in0=z3_ps[:], in1=l3b[:], op=OP.add)
                    sig = tiny.tile([1, 1], F32, tag="sig")
                    nc.scalar.activation(out=sig[:], in_=z3[:], func=AF.Sigmoid)
                    nc.sync.dma_start(out=dout.ap(), in_=sig[:])
            if not tail:
                nc.sync.dma_start(out=dout.ap(), in_=partials[:1, :1])

    nc.finalize()
    return nc


# ------------------------------------------------------------------ driver

_CACHE = {}


def kernel(**inputs):
    in_maps_nc, meta = host_prep(inputs["node_ids"], inputs["edge_index"],
                                 inputs["edge_type"])
    shared = pack_model_inputs(inputs, meta["cnt"])
    in_maps = [{**m, **shared} for m in in_maps_nc]

    key = (meta["Lp"], meta["nv_eq"], tuple(meta["n_k"]))
    if key not in _CACHE:
        _CACHE[key] = build_program(meta)
    nc = _CACHE[key]

    res = run_bass_kernel_spmd(nc, in_maps, core_ids=list(range(NCORES)))
    return np.asarray(res.results[0]["out"], np.float32)
